# revision 8
# baseline (speedup 1.0000x reference)
"""Cross multihead attention (global/local masked head groups) on 8 trn2 cores.

Sharding: core c -> (batch b = c//2, head-group g = c%2).
  g=0: heads 0-7  masked by key_padding_mask[b]
  g=1: heads 8-15 masked by local_mask[b]
Each core computes its group's partial output (attn_out_g @ Wo[:, gs].T)
of shape [T, E]; the host sums the two partials per batch and adds bo.

Key structure:
  - Masked keys contribute nothing (exp -> 0), so the host gathers the
    ~50% unmasked key/value columns into an SQ=640-padded buffer;
    k/v projections, QK^T and AV all shrink accordingly.  Padding
    columns keep a -30000 exp bias so they vanish from the softmax.
  - scoresT orientation [s, t]: mask bias is per-partition for the exp.
  - AV is computed as out[t, d] = expT.T @ [v | ones]: the moving operand
    is only 65 wide (PE cost ~ moving free size), and the ones column
    gives the softmax denominator per t-partition, so normalization is
    a per-partition scalar multiply (no partition broadcast needed).
  - A PE transpose (identity moving operand) flips normalized [t, j]
    head pairs back to [j, t] for the output projection.
  - All big DRAM loads are single batched DMAs (HWDGE serializes at
    ~625ns per dma_start).
  - Software-pipelined emission: the PE stream interleaves "A units"
    (q-proj + QK + exp, which pace at the Act engine's exp speed via
    PSUM buffer rotation) with "B units" (v-proj, AV chains, output
    projection) so the PE never idles while Act catches up.
"""

import os
import sys

sys.path.insert(0, "/opt/trn_rl_repo")

import numpy as np

import concourse.bass as bass
import concourse.mybir as mybir
from concourse.tile import TileContext

B, T, S, E, H = 4, 1024, 1024, 1024, 16
DH = E // H            # 64
HH = H // 2            # 8 heads per group
G = HH * DH            # 512 features per group
SCALING = DH ** -0.5
NEG = -30000.0         # exp(x + NEG) == 0.0 in fp32, no LUT edge cases

F32 = mybir.dt.float32
BF = mybir.dt.bfloat16

ET = E // 128          # 8 contraction tiles
JT = G // 128          # 4 j-tiles (head pairs)
TT = T // 128          # 8 t tiles


def _split_waits(nc):
    """TPB ISA structs hold one sem-wait slot. Tile can emit >1 wait per
    instruction (walrus: 'Too many sync wait commands'); hoist all but the
    last wait onto single-wait NOPs on the same engine, inserted just
    before. Timing is unchanged - the waits would have blocked anyway."""
    k = 0
    for f in nc.m.functions:
        for blk in f.blocks:
            new = []
            for inst in blk.instructions:
                si = inst.sync_info
                w = list(si.on_wait) if si else []
                if len(w) > 1:
                    for wait in w[:-1]:
                        nop = mybir.InstNoOp(name=f"nopw-{k}", ins=[], outs=[])
                        k += 1
                        nop.engine = inst.engine
                        nop.sync_info = mybir.SyncInfo(on_wait=[wait], on_update=[])
                        new.append(nop)
                    inst.sync_info = mybir.SyncInfo(
                        on_wait=[w[-1]], on_update=list(si.on_update)
                    )
                new.append(inst)
            blk.instructions = new
    return nc


def build_nc(st_tiles=5, split=True, off=11):
    ST = st_tiles          # number of 128-wide s tiles after compression
    SQ = ST * 128          # padded compressed key count

    nc = bass.Bass()

    xqT = nc.dram_tensor("xqT", [E, T], BF, kind="ExternalInput")
    xkT = nc.dram_tensor("xkT", [E, SQ], BF, kind="ExternalInput")
    xvT = nc.dram_tensor("xvT", [E, SQ], BF, kind="ExternalInput")
    wqT = nc.dram_tensor("wqT", [E, G], BF, kind="ExternalInput")
    wkT = nc.dram_tensor("wkT", [E, G], BF, kind="ExternalInput")
    wvT = nc.dram_tensor("wvT", [E, G], BF, kind="ExternalInput")
    woT = nc.dram_tensor("woT", [G, E], BF, kind="ExternalInput")
    # packed f32 consts: bq | bk | mb  -> [128, 4 + 4 + ST]
    cf = nc.dram_tensor("cf", [128, 2 * JT + ST], F32, kind="ExternalInput")
    bvr = nc.dram_tensor("bvr", [1, G], BF, kind="ExternalInput")
    idn = nc.dram_tensor("idn", [128, 128], BF, kind="ExternalInput")
    out = nc.dram_tensor("out", [T, E], F32, kind="ExternalOutput")

    with TileContext(nc) as tc:
        with (
            tc.tile_pool(name="const", bufs=1) as pc,
            tc.tile_pool(name="persist", bufs=1) as pp,
            tc.tile_pool(name="exp", bufs=HH * ST) as pe,
            tc.tile_pool(name="small", bufs=4) as psm,
            tc.tile_pool(name="outsb", bufs=2) as po,
            tc.tile_pool(name="ps", bufs=2, space="PSUM") as pps,
        ):
            # ---- staged loads, startup-latency ordered ----
            cf_sb = pc.tile([128, 2 * JT + ST], F32, name="cf_sb")
            nc.sync.dma_start(out=cf_sb[:], in_=cf[:])
            bq_sb = cf_sb[:, 0:JT]
            bk_sb = cf_sb[:, JT:2 * JT]
            mb_sb = cf_sb[:, 2 * JT:]

            # wk/xk land in halves so the k projection starts ~5us earlier
            wk_sb = pc.tile([128, ET * G], BF, name="wk_sb")
            xk_sb = pc.tile([128, ET * SQ], BF, name="xk_sb")
            EH = ET // 2
            for eh in range(2):
                nc.sync.dma_start(
                    out=wk_sb[:].rearrange("p (e t) -> p e t", e=ET)[:, eh * EH:(eh + 1) * EH],
                    in_=wkT[:].rearrange("(e p) t -> p e t", p=128)[:, eh * EH:(eh + 1) * EH],
                )
                nc.sync.dma_start(
                    out=xk_sb[:].rearrange("p (e t) -> p e t", e=ET)[:, eh * EH:(eh + 1) * EH],
                    in_=xkT[:].rearrange("(e p) t -> p e t", p=128)[:, eh * EH:(eh + 1) * EH],
                )
            wq_sb = pc.tile([128, ET * G], BF, name="wq_sb")
            nc.sync.dma_start(
                out=wq_sb[:].rearrange("p (e t) -> p e t", e=ET),
                in_=wqT[:].rearrange("(e p) t -> p e t", p=128),
            )
            xq_sb = pc.tile([128, ET * T], BF, name="xq_sb")
            nc.sync.dma_start(
                out=xq_sb[:].rearrange("p (e t) -> p e t", e=ET),
                in_=xqT[:].rearrange("(e p) t -> p e t", p=128),
            )
            bv_sb = pc.tile([1, G], BF, name="bv_sb")
            nc.sync.dma_start(out=bv_sb[:], in_=bvr[:])
            id_sb = pc.tile([128, 128], BF, name="id_sb")
            nc.sync.dma_start(out=id_sb[:], in_=idn[:])
            wv_sb = pc.tile([128, ET * G], BF, name="wv_sb")
            nc.sync.dma_start(
                out=wv_sb[:].rearrange("p (e t) -> p e t", e=ET),
                in_=wvT[:].rearrange("(e p) t -> p e t", p=128),
            )
            xv_sb = pc.tile([128, ET * SQ], BF, name="xv_sb")
            nc.sync.dma_start(
                out=xv_sb[:].rearrange("p (e t) -> p e t", e=ET),
                in_=xvT[:].rearrange("(e p) t -> p e t", p=128),
            )
            wo_sb = pc.tile([128, JT * E], BF, name="wo_sb")
            nc.sync.dma_start(
                out=wo_sb[:].rearrange("p (r t) -> p r t", r=JT),
                in_=woT[:].rearrange("(r p) t -> p r t", p=128),
            )
            ones_sb = pc.tile([1, 128], BF, name="ones_sb")
            nc.gpsimd.memset(ones_sb[:], 1.0)

            xk3 = xk_sb[:].rearrange("p (e t) -> p e t", e=ET)
            wk3 = wk_sb[:].rearrange("p (e t) -> p e t", e=ET)
            xq3 = xq_sb[:].rearrange("p (e t) -> p e t", e=ET)
            wq3 = wq_sb[:].rearrange("p (e t) -> p e t", e=ET)
            xv3 = xv_sb[:].rearrange("p (e t) -> p e t", e=ET)
            wv3 = wv_sb[:].rearrange("p (e t) -> p e t", e=ET)
            wo3 = wo_sb[:].rearrange("p (r t) -> p r t", r=JT)

            # ---- persistent activations ----
            qT_sb = [pp.tile([128, T], BF, name=f"qT{r}") for r in range(JT)]
            kT_sb = [pp.tile([128, SQ], BF, name=f"kT{r}") for r in range(JT)]
            v_sb = [pp.tile([128, HH * (DH + 1)], BF, name=f"v{st}") for st in range(ST)]
            aT_sb = [pp.tile([128, T], BF, name=f"aT{r}") for r in range(JT)]
            expT = [None] * (HH * ST)

            # ---- k projection (pure PE warmup while other DMAs land) ----
            for r in range(JT):
                ps = pps.tile([128, 1024], F32, tag="qk", name="ps_k")
                for lo, hi in ((0, 512), (512, SQ)):
                    if lo >= SQ:
                        continue
                    for et in range(ET):
                        nc.tensor.matmul(
                            ps[:, lo:hi],
                            lhsT=wk3[:, et, r * 128:(r + 1) * 128],
                            rhs=xk3[:, et, lo:hi],
                            start=(et == 0), stop=(et == ET - 1),
                        )
                nc.vector.tensor_scalar_add(
                    kT_sb[r][:], ps[:, 0:SQ], bk_sb[:, r:r + 1]
                )

            # ---- unit emitters ----
            def emit_q(r):
                ps = pps.tile([128, 1024], F32, tag="qk", name="ps_q")
                for lo, hi in ((0, 512), (512, 1024)):
                    for et in range(ET):
                        nc.tensor.matmul(
                            ps[:, lo:hi],
                            lhsT=wq3[:, et, r * 128:(r + 1) * 128],
                            rhs=xq3[:, et, lo:hi],
                            start=(et == 0), stop=(et == ET - 1),
                        )
                nc.vector.tensor_scalar_add(qT_sb[r][:], ps[:], bq_sb[:, r:r + 1])

            def emit_qk(r, hl, st):
                h = 2 * r + hl
                po_ = hl * DH
                ps_s = pps.tile([128, 1024], F32, tag="qk", name="ps_s")
                for lo, hi in ((0, 512), (512, 1024)):
                    nc.tensor.matmul(
                        ps_s[:, lo:hi],
                        lhsT=kT_sb[r][po_:po_ + DH, st * 128:(st + 1) * 128],
                        rhs=qT_sb[r][po_:po_ + DH, lo:hi],
                        start=True, stop=True,
                    )
                ex = pe.tile([128, T], BF, tag="exp", name=f"exp{h}_{st}")
                expT[h * ST + st] = ex
                nc.scalar.activation(
                    ex[:], ps_s[:],
                    mybir.ActivationFunctionType.Exp,
                    bias=mb_sb[:, st:st + 1], scale=SCALING,
                )

            def emit_v(st):
                ps = pps.tile([128, 1024], F32, tag="qk", name="ps_v")
                for et in range(ET):
                    nc.tensor.matmul(
                        ps[:, 0:G],
                        lhsT=xv3[:, et, st * 128:(st + 1) * 128],
                        rhs=wv3[:, et, :],
                        start=(et == 0), stop=False,
                    )
                nc.tensor.matmul(  # += ones[1,128].T @ bv[1,512]
                    ps[:, 0:G], lhsT=ones_sb[:], rhs=bv_sb[:],
                    start=False, stop=True,
                )
                v3 = v_sb[st][:].rearrange("p (h x) -> p h x", x=DH + 1)
                nc.vector.tensor_copy(
                    v3[:, :, 0:DH], ps[:, 0:G].rearrange("p (h x) -> p h x", x=DH)
                )
                nc.gpsimd.memset(v3[:, :, DH:DH + 1], 1.0)

            pav_cur = [None]

            def emit_av(hp, tt, i):
                h = 2 * hp + i
                if i == 0:
                    pav_cur[0] = pps.tile([128, 1024], F32, tag="avp",
                                          name="pav", bufs=2)
                pav = pav_cur[0]
                base = i * 512
                for st in range(ST):
                    nc.tensor.matmul(
                        pav[:, base:base + DH + 1],
                        lhsT=expT[h * ST + st][:, tt * 128:(tt + 1) * 128],
                        rhs=v_sb[st][:, h * (DH + 1):(h + 1) * (DH + 1)],
                        start=(st == 0), stop=(st == ST - 1),
                    )
                if i == 1:
                    rec = psm.tile([128, 2], F32, tag="rec", name="rec")
                    nc.vector.reciprocal(
                        rec[:],
                        pav[:].rearrange("p (b x) -> p b x", b=2)[:, :, DH:DH + 1],
                    )
                    op = psm.tile([128, 128], BF, tag="op", name="op")
                    nc.vector.tensor_scalar_mul(
                        op[:, 0:DH], pav[:, 0:DH], rec[:, 0:1]
                    )
                    nc.vector.tensor_scalar_mul(
                        op[:, DH:2 * DH], pav[:, 512:512 + DH], rec[:, 1:2]
                    )
                    tr = pps.tile([128, 1024], BF, tag="qk", name="tr")
                    nc.tensor.transpose(tr[:, 0:128], op[:], id_sb[:])
                    nc.vector.tensor_copy(
                        aT_sb[hp][:, tt * 128:(tt + 1) * 128], tr[:, 0:128]
                    )

            def emit_op(t2):
                osb = po.tile([128, E], F32, tag="osb", name="osb")
                for oc in range(2):
                    pu = pps.tile([128, 1024], F32, tag="qk", name="pu")
                    for r in range(JT):
                        nc.tensor.matmul(
                            pu[:, 0:512],
                            lhsT=aT_sb[r][:, t2 * 128:(t2 + 1) * 128],
                            rhs=wo3[:, r, oc * 512:(oc + 1) * 512],
                            start=(r == 0), stop=(r == JT - 1),
                        )
                    nc.scalar.copy(osb[:, oc * 512:(oc + 1) * 512], pu[:, 0:512])
                nc.sync.dma_start(
                    out=out[t2 * 128:(t2 + 1) * 128, :], in_=osb[:]
                )

            # ---- interleaved A/B emission ----
            a_list = []
            for r in range(JT):
                a_list.append(("q", r))
                for hl in range(2):
                    for st in range(ST):
                        a_list.append(("qk", r, hl, st))
            b_list = [("v", st) for st in range(ST)]
            for hp in range(JT):
                for tt in range(TT):
                    b_list.append(("av", hp, tt, 0))
                    b_list.append(("av", hp, tt, 1))
                    if hp == JT - 1 and tt >= 1:
                        b_list.append(("op", tt - 1))
            b_list.append(("op", TT - 1))

            def emit(u):
                kind = u[0]
                if kind == "q":
                    emit_q(u[1])
                elif kind == "qk":
                    emit_qk(u[1], u[2], u[3])
                elif kind == "v":
                    emit_v(u[1])
                elif kind == "av":
                    emit_av(u[1], u[2], u[3])
                elif kind == "op":
                    emit_op(u[1])

            def a_idx(r, hl, st):
                return r * (2 * ST + 1) + 1 + hl * ST + st

            def prereq(u):
                if u[0] == "av":
                    _, hp, tt, i = u
                    return a_idx(hp, i, ST - 1)
                if u[0] == "op":
                    return a_idx(JT - 1, 1, ST - 1)
                return 0

            nA, nB = len(a_list), len(b_list)
            ib = 0
            for i, u in enumerate(a_list):
                emit(u)
                if i + 1 > off:
                    tgt = ((i + 1 - off) * nB + (nA - off) - 1) // (nA - off)
                    while ib < min(tgt, nB) and prereq(b_list[ib]) <= i:
                        emit(b_list[ib])
                        ib += 1
            while ib < nB:
                emit(b_list[ib])
                ib += 1
    return _split_waits(nc) if split else nc


_NC_CACHE = {}


def _get_nc(st_tiles=5):
    if st_tiles not in _NC_CACHE:
        _NC_CACHE[st_tiles] = build_nc(st_tiles)
    return _NC_CACHE[st_tiles]


def make_in_maps(query, key, value, key_padding_mask, local_mask,
                 Wq, bq, Wk, bk, Wv, bv, Wo, bo, st_tiles=5):
    import ml_dtypes
    f = np.float32
    bf = ml_dtypes.bfloat16
    SQ = st_tiles * 128
    ident = np.eye(128, dtype=bf)
    in_maps = []
    for c in range(8):
        b, g = c // 2, c % 2
        gs = slice(g * G, (g + 1) * G)
        mask = np.asarray((key_padding_mask if g == 0 else local_mask)[b])
        sel = np.flatnonzero(~mask)
        ns = sel.size
        assert ns <= SQ, (ns, SQ)
        xk = np.zeros((E, SQ), dtype=bf)
        xk[:, :ns] = np.asarray(key[b]).T[:, sel]
        xv = np.zeros((E, SQ), dtype=bf)
        xv[:, :ns] = np.asarray(value[b]).T[:, sel]
        mbias = np.full(SQ, NEG, f)
        mbias[:ns] = 0.0
        cfm = np.concatenate([
            np.asarray(bq)[gs].astype(f).reshape(JT, 128).T,
            np.asarray(bk)[gs].astype(f).reshape(JT, 128).T,
            mbias.reshape(st_tiles, 128).T,
        ], axis=1)
        in_maps.append({
            "xqT": np.ascontiguousarray(np.asarray(query[b]).T, dtype=bf),
            "xkT": xk,
            "xvT": xv,
            "wqT": np.ascontiguousarray(np.asarray(Wq)[gs, :].T, dtype=bf),
            "wkT": np.ascontiguousarray(np.asarray(Wk)[gs, :].T, dtype=bf),
            "wvT": np.ascontiguousarray(np.asarray(Wv)[gs, :].T, dtype=bf),
            "woT": np.ascontiguousarray(np.asarray(Wo)[:, gs].T, dtype=bf),
            "cf": np.ascontiguousarray(cfm),
            "bvr": np.ascontiguousarray(np.asarray(bv)[gs].astype(bf).reshape(1, G)),
            "idn": ident,
        })
    return in_maps


def _needed_st_tiles(key_padding_mask, local_mask):
    worst = 0
    for c in range(8):
        b, g = c // 2, c % 2
        mask = np.asarray((key_padding_mask if g == 0 else local_mask)[b])
        worst = max(worst, int((~mask).sum()))
    return max(1, -(-worst // 128))


def kernel(query, key, value, key_padding_mask, local_mask,
           Wq, bq, Wk, bk, Wv, bv, Wo, bo, _trace=False, _tmpdir=None):
    from concourse.bass_utils import run_bass_kernel_spmd

    st = min(max(_needed_st_tiles(key_padding_mask, local_mask), 5), 8)
    nc = _get_nc(st)
    in_maps = make_in_maps(query, key, value, key_padding_mask, local_mask,
                           Wq, bq, Wk, bk, Wv, bv, Wo, bo, st_tiles=st)
    try:
        res = run_bass_kernel_spmd(nc, in_maps, list(range(8)),
                                   trace=_trace, tmpdir=_tmpdir)
    except Exception:
        # transient device/transport failures have been observed on the
        # axon path; one fresh attempt is cheap relative to a hard fail
        res = run_bass_kernel_spmd(nc, in_maps, list(range(8)),
                                   trace=_trace, tmpdir=_tmpdir)
    outs = [np.asarray(r["out"]) for r in res.results]
    full = np.stack([outs[2 * b] + outs[2 * b + 1] for b in range(B)])
    full += np.asarray(bo, dtype=np.float32)
    if _trace:
        kernel._last_exec_time_ns = res.exec_time_ns
        kernel._last_profile = res.profile_json
    return full.astype(np.float32)


# revision 9
# speedup vs baseline: 1.1088x; 1.1088x over previous
"""Cross multihead attention (global/local masked head groups) on 8 trn2 cores.

Sharding: core c -> (batch b = c//2, head-group g = c%2).
  g=0: heads 0-7  masked by key_padding_mask[b]
  g=1: heads 8-15 masked by local_mask[b]
Each core computes its group's partial output (attn_out_g @ Wo[:, gs].T)
of shape [T, E]; the host sums the two partials per batch and adds bo.

Key structure:
  - Masked keys contribute nothing (exp -> 0), so the host gathers the
    ~50% unmasked key/value columns into an SQ=640-padded buffer;
    k/v projections, QK^T and AV all shrink accordingly.  Padding
    columns keep a -30000 exp bias so they vanish from the softmax.
  - scoresT orientation [s, t]: mask bias is per-partition for the exp.
  - AV is computed as out[t, d] = expT.T @ [v | ones]: the moving operand
    is only 65 wide (PE cost ~ moving free size), and the ones column
    gives the softmax denominator per t-partition, so normalization is
    a per-partition scalar multiply (no partition broadcast needed).
  - A PE transpose (identity moving operand) flips normalized [t, j]
    head pairs back to [j, t] for the output projection.
  - All big DRAM loads are single batched DMAs (HWDGE serializes at
    ~625ns per dma_start).
  - Software-pipelined emission: the PE stream interleaves "A units"
    (q-proj + QK + exp, which pace at the Act engine's exp speed via
    PSUM buffer rotation) with "B units" (v-proj, AV chains, output
    projection) so the PE never idles while Act catches up.
"""

import os
import sys

sys.path.insert(0, "/opt/trn_rl_repo")

import numpy as np

import concourse.bass as bass
import concourse.mybir as mybir
from concourse.tile import TileContext

B, T, S, E, H = 4, 1024, 1024, 1024, 16
DH = E // H            # 64
HH = H // 2            # 8 heads per group
G = HH * DH            # 512 features per group
SCALING = DH ** -0.5
NEG = -30000.0         # exp(x + NEG) == 0.0 in fp32, no LUT edge cases

F32 = mybir.dt.float32
BF = mybir.dt.bfloat16

ET = E // 128          # 8 contraction tiles
JT = G // 128          # 4 j-tiles (head pairs)
TT = T // 128          # 8 t tiles


def _split_waits(nc):
    """TPB ISA structs hold one sem-wait slot. Tile can emit >1 wait per
    instruction (walrus: 'Too many sync wait commands'); hoist all but the
    last wait onto single-wait NOPs on the same engine, inserted just
    before. Timing is unchanged - the waits would have blocked anyway."""
    k = 0
    for f in nc.m.functions:
        for blk in f.blocks:
            new = []
            for inst in blk.instructions:
                si = inst.sync_info
                w = list(si.on_wait) if si else []
                if len(w) > 1:
                    for wait in w[:-1]:
                        nop = mybir.InstNoOp(name=f"nopw-{k}", ins=[], outs=[])
                        k += 1
                        nop.engine = inst.engine
                        nop.sync_info = mybir.SyncInfo(on_wait=[wait], on_update=[])
                        new.append(nop)
                    inst.sync_info = mybir.SyncInfo(
                        on_wait=[w[-1]], on_update=list(si.on_update)
                    )
                new.append(inst)
            blk.instructions = new
    return nc


def build_nc(st_tiles=5, split=True, off=11):
    ST = st_tiles          # number of 128-wide s tiles after compression
    SQ = ST * 128          # padded compressed key count

    nc = bass.Bass()

    xqT = nc.dram_tensor("xqT", [E, T], BF, kind="ExternalInput")
    xkT = nc.dram_tensor("xkT", [E, SQ], BF, kind="ExternalInput")
    xvT = nc.dram_tensor("xvT", [E, SQ], BF, kind="ExternalInput")
    wqT = nc.dram_tensor("wqT", [E, G], BF, kind="ExternalInput")
    wkT = nc.dram_tensor("wkT", [E, G], BF, kind="ExternalInput")
    wvT = nc.dram_tensor("wvT", [E, G], BF, kind="ExternalInput")
    woT = nc.dram_tensor("woT", [G, E], BF, kind="ExternalInput")
    # packed f32 consts: bq | bk | mb  -> [128, 4 + 4 + ST]
    cf = nc.dram_tensor("cf", [128, 2 * JT + ST], F32, kind="ExternalInput")
    bvr = nc.dram_tensor("bvr", [1, G], BF, kind="ExternalInput")
    idn = nc.dram_tensor("idn", [128, 128], BF, kind="ExternalInput")
    out = nc.dram_tensor("out", [T, E], F32, kind="ExternalOutput")

    with TileContext(nc) as tc:
        with (
            tc.tile_pool(name="const", bufs=1) as pc,
            tc.tile_pool(name="persist", bufs=1) as pp,
            tc.tile_pool(name="exp", bufs=HH * ST) as pe,
            tc.tile_pool(name="small", bufs=4) as psm,
            tc.tile_pool(name="outsb", bufs=2) as po,
            tc.tile_pool(name="ps", bufs=2, space="PSUM") as pps,
        ):
            # ---- staged loads, startup-latency ordered ----
            cf_sb = pc.tile([128, 2 * JT + ST], F32, name="cf_sb")
            nc.sync.dma_start(out=cf_sb[:], in_=cf[:])
            bq_sb = cf_sb[:, 0:JT]
            bk_sb = cf_sb[:, JT:2 * JT]
            mb_sb = cf_sb[:, 2 * JT:]

            # wk/xk land in halves so the k projection starts ~5us earlier
            wk_sb = pc.tile([128, ET * G], BF, name="wk_sb")
            xk_sb = pc.tile([128, ET * SQ], BF, name="xk_sb")
            EH = ET // 2
            for eh in range(2):
                nc.sync.dma_start(
                    out=wk_sb[:].rearrange("p (e t) -> p e t", e=ET)[:, eh * EH:(eh + 1) * EH],
                    in_=wkT[:].rearrange("(e p) t -> p e t", p=128)[:, eh * EH:(eh + 1) * EH],
                )
                nc.sync.dma_start(
                    out=xk_sb[:].rearrange("p (e t) -> p e t", e=ET)[:, eh * EH:(eh + 1) * EH],
                    in_=xkT[:].rearrange("(e p) t -> p e t", p=128)[:, eh * EH:(eh + 1) * EH],
                )
            wq_sb = pc.tile([128, ET * G], BF, name="wq_sb")
            nc.sync.dma_start(
                out=wq_sb[:].rearrange("p (e t) -> p e t", e=ET),
                in_=wqT[:].rearrange("(e p) t -> p e t", p=128),
            )
            xq_sb = pc.tile([128, ET * T], BF, name="xq_sb")
            nc.sync.dma_start(
                out=xq_sb[:].rearrange("p (e t) -> p e t", e=ET),
                in_=xqT[:].rearrange("(e p) t -> p e t", p=128),
            )
            bv_sb = pc.tile([1, G], BF, name="bv_sb")
            nc.sync.dma_start(out=bv_sb[:], in_=bvr[:])
            id_sb = pc.tile([128, 128], BF, name="id_sb")
            nc.sync.dma_start(out=id_sb[:], in_=idn[:])
            wv_sb = pc.tile([128, ET * G], BF, name="wv_sb")
            nc.sync.dma_start(
                out=wv_sb[:].rearrange("p (e t) -> p e t", e=ET),
                in_=wvT[:].rearrange("(e p) t -> p e t", p=128),
            )
            xv_sb = pc.tile([128, ET * SQ], BF, name="xv_sb")
            nc.sync.dma_start(
                out=xv_sb[:].rearrange("p (e t) -> p e t", e=ET),
                in_=xvT[:].rearrange("(e p) t -> p e t", p=128),
            )
            wo_sb = pc.tile([128, JT * E], BF, name="wo_sb")
            nc.sync.dma_start(
                out=wo_sb[:].rearrange("p (r t) -> p r t", r=JT),
                in_=woT[:].rearrange("(r p) t -> p r t", p=128),
            )
            ones_sb = pc.tile([1, 128], BF, name="ones_sb")
            nc.gpsimd.memset(ones_sb[:], 1.0)

            xk3 = xk_sb[:].rearrange("p (e t) -> p e t", e=ET)
            wk3 = wk_sb[:].rearrange("p (e t) -> p e t", e=ET)
            xq3 = xq_sb[:].rearrange("p (e t) -> p e t", e=ET)
            wq3 = wq_sb[:].rearrange("p (e t) -> p e t", e=ET)
            xv3 = xv_sb[:].rearrange("p (e t) -> p e t", e=ET)
            wv3 = wv_sb[:].rearrange("p (e t) -> p e t", e=ET)
            wo3 = wo_sb[:].rearrange("p (r t) -> p r t", r=JT)

            # ---- persistent activations ----
            qT_sb = [pp.tile([128, T], BF, name=f"qT{r}") for r in range(JT)]
            kT_sb = [pp.tile([128, SQ], BF, name=f"kT{r}") for r in range(JT)]
            v_sb = [pp.tile([128, HH * (DH + 1)], BF, name=f"v{st}") for st in range(ST)]
            aT_sb = [pp.tile([128, T], BF, name=f"aT{r}") for r in range(JT)]
            expT = [None] * (HH * ST)

            # ---- k projection (pure PE warmup while other DMAs land) ----
            for r in range(JT):
                ps = pps.tile([128, 1024], F32, tag="qk", name="ps_k")
                for lo, hi in ((0, 512), (512, SQ)):
                    if lo >= SQ:
                        continue
                    for et in range(ET):
                        nc.tensor.matmul(
                            ps[:, lo:hi],
                            lhsT=wk3[:, et, r * 128:(r + 1) * 128],
                            rhs=xk3[:, et, lo:hi],
                            start=(et == 0), stop=(et == ET - 1),
                        )
                nc.vector.tensor_scalar_add(
                    kT_sb[r][:], ps[:, 0:SQ], bk_sb[:, r:r + 1]
                )

            # ---- unit emitters ----
            def emit_q(r):
                ps = pps.tile([128, 1024], F32, tag="qk", name="ps_q")
                for lo, hi in ((0, 512), (512, 1024)):
                    for et in range(ET):
                        nc.tensor.matmul(
                            ps[:, lo:hi],
                            lhsT=wq3[:, et, r * 128:(r + 1) * 128],
                            rhs=xq3[:, et, lo:hi],
                            start=(et == 0), stop=(et == ET - 1),
                        )
                nc.vector.tensor_scalar_add(qT_sb[r][:], ps[:], bq_sb[:, r:r + 1])

            def emit_qk(r, hl, st):
                h = 2 * r + hl
                po_ = hl * DH
                ps_s = pps.tile([128, 1024], F32, tag="qk", name="ps_s")
                for lo, hi in ((0, 512), (512, 1024)):
                    nc.tensor.matmul(
                        ps_s[:, lo:hi],
                        lhsT=kT_sb[r][po_:po_ + DH, st * 128:(st + 1) * 128],
                        rhs=qT_sb[r][po_:po_ + DH, lo:hi],
                        start=True, stop=True,
                    )
                ex = pe.tile([128, T], BF, tag="exp", name=f"exp{h}_{st}")
                expT[h * ST + st] = ex
                nc.scalar.activation(
                    ex[:], ps_s[:],
                    mybir.ActivationFunctionType.Exp,
                    bias=mb_sb[:, st:st + 1], scale=SCALING,
                )

            def emit_v(st):
                ps = pps.tile([128, 1024], F32, tag="qk", name="ps_v")
                for et in range(ET):
                    nc.tensor.matmul(
                        ps[:, 0:G],
                        lhsT=xv3[:, et, st * 128:(st + 1) * 128],
                        rhs=wv3[:, et, :],
                        start=(et == 0), stop=False,
                    )
                nc.tensor.matmul(  # += ones[1,128].T @ bv[1,512]
                    ps[:, 0:G], lhsT=ones_sb[:], rhs=bv_sb[:],
                    start=False, stop=True,
                )
                v3 = v_sb[st][:].rearrange("p (h x) -> p h x", x=DH + 1)
                nc.vector.tensor_copy(
                    v3[:, :, 0:DH], ps[:, 0:G].rearrange("p (h x) -> p h x", x=DH)
                )
                nc.gpsimd.memset(v3[:, :, DH:DH + 1], 1.0)

            pav_cur = [None]

            def emit_av(hp, tt, i):
                h = 2 * hp + i
                if i == 0:
                    pav_cur[0] = pps.tile([128, 1024], F32, tag="avp",
                                          name="pav", bufs=1)
                pav = pav_cur[0]
                base = i * 512
                for st in range(ST):
                    nc.tensor.matmul(
                        pav[:, base:base + DH + 1],
                        lhsT=expT[h * ST + st][:, tt * 128:(tt + 1) * 128],
                        rhs=v_sb[st][:, h * (DH + 1):(h + 1) * (DH + 1)],
                        start=(st == 0), stop=(st == ST - 1),
                    )
                if i == 1:
                    rec = psm.tile([128, 2], F32, tag="rec", name="rec")
                    nc.vector.reciprocal(
                        rec[:],
                        pav[:].rearrange("p (b x) -> p b x", b=2)[:, :, DH:DH + 1],
                    )
                    op = psm.tile([128, 128], BF, tag="op", name="op")
                    nc.vector.tensor_scalar_mul(
                        op[:, 0:DH], pav[:, 0:DH], rec[:, 0:1]
                    )
                    nc.vector.tensor_scalar_mul(
                        op[:, DH:2 * DH], pav[:, 512:512 + DH], rec[:, 1:2]
                    )
                    tr = pps.tile([128, 128], BF, tag="tr", name="tr", bufs=2)
                    nc.tensor.transpose(tr[:], op[:], id_sb[:])
                    nc.vector.tensor_copy(
                        aT_sb[hp][:, tt * 128:(tt + 1) * 128], tr[:]
                    )

            def emit_op(t2):
                osb = po.tile([128, E], F32, tag="osb", name="osb")
                for oc in range(2):
                    pu = pps.tile([128, 1024], F32, tag="qk", name="pu")
                    for r in range(JT):
                        nc.tensor.matmul(
                            pu[:, 0:512],
                            lhsT=aT_sb[r][:, t2 * 128:(t2 + 1) * 128],
                            rhs=wo3[:, r, oc * 512:(oc + 1) * 512],
                            start=(r == 0), stop=(r == JT - 1),
                        )
                    nc.scalar.copy(osb[:, oc * 512:(oc + 1) * 512], pu[:, 0:512])
                nc.sync.dma_start(
                    out=out[t2 * 128:(t2 + 1) * 128, :], in_=osb[:]
                )

            # ---- interleaved A/B emission ----
            a_list = []
            for r in range(JT):
                a_list.append(("q", r))
                for hl in range(2):
                    for st in range(ST):
                        a_list.append(("qk", r, hl, st))
            b_list = [("v", st) for st in range(ST)]
            for hp in range(JT):
                for tt in range(TT):
                    b_list.append(("av", hp, tt, 0))
                    b_list.append(("av", hp, tt, 1))
                    if hp == JT - 1 and tt >= 1:
                        b_list.append(("op", tt - 1))
            b_list.append(("op", TT - 1))

            def emit(u):
                kind = u[0]
                if kind == "q":
                    emit_q(u[1])
                elif kind == "qk":
                    emit_qk(u[1], u[2], u[3])
                elif kind == "v":
                    emit_v(u[1])
                elif kind == "av":
                    emit_av(u[1], u[2], u[3])
                elif kind == "op":
                    emit_op(u[1])

            def a_idx(r, hl, st):
                return r * (2 * ST + 1) + 1 + hl * ST + st

            def prereq(u):
                if u[0] == "av":
                    _, hp, tt, i = u
                    return a_idx(hp, i, ST - 1)
                if u[0] == "op":
                    return a_idx(JT - 1, 1, ST - 1)
                return 0

            nA, nB = len(a_list), len(b_list)
            ib = 0
            for i, u in enumerate(a_list):
                emit(u)
                if i + 1 > off:
                    tgt = ((i + 1 - off) * nB + (nA - off) - 1) // (nA - off)
                    while ib < min(tgt, nB) and prereq(b_list[ib]) <= i:
                        emit(b_list[ib])
                        ib += 1
            while ib < nB:
                emit(b_list[ib])
                ib += 1
    return _split_waits(nc) if split else nc


_NC_CACHE = {}


def _get_nc(st_tiles=5):
    if st_tiles not in _NC_CACHE:
        _NC_CACHE[st_tiles] = build_nc(st_tiles)
    return _NC_CACHE[st_tiles]


def make_in_maps(query, key, value, key_padding_mask, local_mask,
                 Wq, bq, Wk, bk, Wv, bv, Wo, bo, st_tiles=5):
    import ml_dtypes
    f = np.float32
    bf = ml_dtypes.bfloat16
    SQ = st_tiles * 128
    ident = np.eye(128, dtype=bf)
    in_maps = []
    for c in range(8):
        b, g = c // 2, c % 2
        gs = slice(g * G, (g + 1) * G)
        mask = np.asarray((key_padding_mask if g == 0 else local_mask)[b])
        sel = np.flatnonzero(~mask)
        ns = sel.size
        assert ns <= SQ, (ns, SQ)
        xk = np.zeros((E, SQ), dtype=bf)
        xk[:, :ns] = np.asarray(key[b]).T[:, sel]
        xv = np.zeros((E, SQ), dtype=bf)
        xv[:, :ns] = np.asarray(value[b]).T[:, sel]
        mbias = np.full(SQ, NEG, f)
        mbias[:ns] = 0.0
        cfm = np.concatenate([
            np.asarray(bq)[gs].astype(f).reshape(JT, 128).T,
            np.asarray(bk)[gs].astype(f).reshape(JT, 128).T,
            mbias.reshape(st_tiles, 128).T,
        ], axis=1)
        in_maps.append({
            "xqT": np.ascontiguousarray(np.asarray(query[b]).T, dtype=bf),
            "xkT": xk,
            "xvT": xv,
            "wqT": np.ascontiguousarray(np.asarray(Wq)[gs, :].T, dtype=bf),
            "wkT": np.ascontiguousarray(np.asarray(Wk)[gs, :].T, dtype=bf),
            "wvT": np.ascontiguousarray(np.asarray(Wv)[gs, :].T, dtype=bf),
            "woT": np.ascontiguousarray(np.asarray(Wo)[:, gs].T, dtype=bf),
            "cf": np.ascontiguousarray(cfm),
            "bvr": np.ascontiguousarray(np.asarray(bv)[gs].astype(bf).reshape(1, G)),
            "idn": ident,
        })
    return in_maps


def _needed_st_tiles(key_padding_mask, local_mask):
    worst = 0
    for c in range(8):
        b, g = c // 2, c % 2
        mask = np.asarray((key_padding_mask if g == 0 else local_mask)[b])
        worst = max(worst, int((~mask).sum()))
    return max(1, -(-worst // 128))


def kernel(query, key, value, key_padding_mask, local_mask,
           Wq, bq, Wk, bk, Wv, bv, Wo, bo, _trace=False, _tmpdir=None):
    from concourse.bass_utils import run_bass_kernel_spmd

    st = min(max(_needed_st_tiles(key_padding_mask, local_mask), 5), 8)
    nc = _get_nc(st)
    in_maps = make_in_maps(query, key, value, key_padding_mask, local_mask,
                           Wq, bq, Wk, bk, Wv, bv, Wo, bo, st_tiles=st)
    try:
        res = run_bass_kernel_spmd(nc, in_maps, list(range(8)),
                                   trace=_trace, tmpdir=_tmpdir)
    except Exception:
        # transient device/transport failures have been observed on the
        # axon path; one fresh attempt is cheap relative to a hard fail
        res = run_bass_kernel_spmd(nc, in_maps, list(range(8)),
                                   trace=_trace, tmpdir=_tmpdir)
    outs = [np.asarray(r["out"]) for r in res.results]
    full = np.stack([outs[2 * b] + outs[2 * b + 1] for b in range(B)])
    full += np.asarray(bo, dtype=np.float32)
    if _trace:
        kernel._last_exec_time_ns = res.exec_time_ns
        kernel._last_profile = res.profile_json
    return full.astype(np.float32)


# revision 10
# speedup vs baseline: 1.1603x; 1.0464x over previous
"""Cross multihead attention (global/local masked head groups) on 8 trn2 cores.

Sharding: core c -> (batch b = c//2, head-group g = c%2).
  g=0: heads 0-7  masked by key_padding_mask[b]
  g=1: heads 8-15 masked by local_mask[b]
Each core computes its group's partial output (attn_out_g @ Wo[:, gs].T)
of shape [T, E]; the host sums the two partials per batch and adds bo.

Key structure:
  - Masked keys contribute nothing (exp -> 0), so the host gathers the
    ~50% unmasked key/value columns into an SQ=640-padded buffer;
    k/v projections, QK^T and AV all shrink accordingly.  Padding
    columns keep a -30000 exp bias so they vanish from the softmax.
  - scoresT orientation [s, t]: mask bias is per-partition for the exp.
  - AV is computed as out[t, d] = expT.T @ [v | ones]: the moving operand
    is only 65 wide (PE cost ~ moving free size), and the ones column
    gives the softmax denominator per t-partition, so normalization is
    a per-partition scalar multiply (no partition broadcast needed).
  - A PE transpose (identity moving operand) flips normalized [t, j]
    head pairs back to [j, t] for the output projection.
  - All big DRAM loads are single batched DMAs (HWDGE serializes at
    ~625ns per dma_start).
  - Software-pipelined emission: the PE stream interleaves "A units"
    (q-proj + QK + exp, which pace at the Act engine's exp speed via
    PSUM buffer rotation) with "B units" (v-proj, AV chains, output
    projection) so the PE never idles while Act catches up.
"""

import os
import sys

sys.path.insert(0, "/opt/trn_rl_repo")

import numpy as np

import concourse.bass as bass
import concourse.mybir as mybir
from concourse.tile import TileContext

B, T, S, E, H = 4, 1024, 1024, 1024, 16
DH = E // H            # 64
HH = H // 2            # 8 heads per group
G = HH * DH            # 512 features per group
SCALING = DH ** -0.5
NEG = -30000.0         # exp(x + NEG) == 0.0 in fp32, no LUT edge cases

F32 = mybir.dt.float32
BF = mybir.dt.bfloat16

ET = E // 128          # 8 contraction tiles
JT = G // 128          # 4 j-tiles (head pairs)
TT = T // 128          # 8 t tiles


def _split_waits(nc):
    """TPB ISA structs hold one sem-wait slot. Tile can emit >1 wait per
    instruction (walrus: 'Too many sync wait commands'); hoist all but the
    last wait onto single-wait NOPs on the same engine, inserted just
    before. Timing is unchanged - the waits would have blocked anyway."""
    k = 0
    for f in nc.m.functions:
        for blk in f.blocks:
            new = []
            for inst in blk.instructions:
                si = inst.sync_info
                w = list(si.on_wait) if si else []
                if len(w) > 1:
                    for wait in w[:-1]:
                        nop = mybir.InstNoOp(name=f"nopw-{k}", ins=[], outs=[])
                        k += 1
                        nop.engine = inst.engine
                        nop.sync_info = mybir.SyncInfo(on_wait=[wait], on_update=[])
                        new.append(nop)
                    inst.sync_info = mybir.SyncInfo(
                        on_wait=[w[-1]], on_update=list(si.on_update)
                    )
                new.append(inst)
            blk.instructions = new
    return nc


def build_nc(st_tiles=5, split=True, off=11):
    ST = st_tiles          # number of 128-wide s tiles after compression
    SQ = ST * 128          # padded compressed key count

    nc = bass.Bass()

    xqT = nc.dram_tensor("xqT", [E, T], BF, kind="ExternalInput")
    xkT = nc.dram_tensor("xkT", [E, SQ], BF, kind="ExternalInput")
    xvT = nc.dram_tensor("xvT", [E, SQ], BF, kind="ExternalInput")
    wqT = nc.dram_tensor("wqT", [E, G], BF, kind="ExternalInput")
    wkT = nc.dram_tensor("wkT", [E, G], BF, kind="ExternalInput")
    wvT = nc.dram_tensor("wvT", [E, G], BF, kind="ExternalInput")
    woT = nc.dram_tensor("woT", [G, E], BF, kind="ExternalInput")
    # packed f32 consts: bq | bk | mb  -> [128, 4 + 4 + ST]
    cf = nc.dram_tensor("cf", [128, 2 * JT + ST], F32, kind="ExternalInput")
    bvr = nc.dram_tensor("bvr", [1, G], BF, kind="ExternalInput")
    idn = nc.dram_tensor("idn", [128, 128], BF, kind="ExternalInput")
    out = nc.dram_tensor("out", [T, E], F32, kind="ExternalOutput")

    with TileContext(nc) as tc:
        with (
            tc.tile_pool(name="const", bufs=1) as pc,
            tc.tile_pool(name="persist", bufs=1) as pp,
            tc.tile_pool(name="exp", bufs=HH * ST) as pe,
            tc.tile_pool(name="small", bufs=4) as psm,
            tc.tile_pool(name="outsb", bufs=2) as po,
            tc.tile_pool(name="ps", bufs=2, space="PSUM") as pps,
        ):
            # ---- staged loads, startup-latency ordered ----
            cf_sb = pc.tile([128, 2 * JT + ST], F32, name="cf_sb")
            nc.sync.dma_start(out=cf_sb[:], in_=cf[:])
            bq_sb = cf_sb[:, 0:JT]
            bk_sb = cf_sb[:, JT:2 * JT]
            mb_sb = cf_sb[:, 2 * JT:]

            # wk/xk land in halves so the k projection starts ~5us earlier
            wk_sb = pc.tile([128, ET * G], BF, name="wk_sb")
            xk_sb = pc.tile([128, ET * SQ], BF, name="xk_sb")
            EH = ET // 2
            for eh in range(2):
                nc.sync.dma_start(
                    out=wk_sb[:].rearrange("p (e t) -> p e t", e=ET)[:, eh * EH:(eh + 1) * EH],
                    in_=wkT[:].rearrange("(e p) t -> p e t", p=128)[:, eh * EH:(eh + 1) * EH],
                )
                nc.sync.dma_start(
                    out=xk_sb[:].rearrange("p (e t) -> p e t", e=ET)[:, eh * EH:(eh + 1) * EH],
                    in_=xkT[:].rearrange("(e p) t -> p e t", p=128)[:, eh * EH:(eh + 1) * EH],
                )
            wq_sb = pc.tile([128, ET * G], BF, name="wq_sb")
            nc.sync.dma_start(
                out=wq_sb[:].rearrange("p (e t) -> p e t", e=ET),
                in_=wqT[:].rearrange("(e p) t -> p e t", p=128),
            )
            xq_sb = pc.tile([128, ET * T], BF, name="xq_sb")
            nc.sync.dma_start(
                out=xq_sb[:].rearrange("p (e t) -> p e t", e=ET),
                in_=xqT[:].rearrange("(e p) t -> p e t", p=128),
            )
            bv_sb = pc.tile([1, G], BF, name="bv_sb")
            nc.sync.dma_start(out=bv_sb[:], in_=bvr[:])
            id_sb = pc.tile([128, 128], BF, name="id_sb")
            nc.sync.dma_start(out=id_sb[:], in_=idn[:])
            wv_sb = pc.tile([128, ET * G], BF, name="wv_sb")
            nc.sync.dma_start(
                out=wv_sb[:].rearrange("p (e t) -> p e t", e=ET),
                in_=wvT[:].rearrange("(e p) t -> p e t", p=128),
            )
            xv_sb = pc.tile([128, ET * SQ], BF, name="xv_sb")
            nc.sync.dma_start(
                out=xv_sb[:].rearrange("p (e t) -> p e t", e=ET),
                in_=xvT[:].rearrange("(e p) t -> p e t", p=128),
            )
            wo_sb = pc.tile([128, JT * E], BF, name="wo_sb")
            nc.sync.dma_start(
                out=wo_sb[:].rearrange("p (r t) -> p r t", r=JT),
                in_=woT[:].rearrange("(r p) t -> p r t", p=128),
            )
            ones_sb = pc.tile([1, 128], BF, name="ones_sb")
            nc.gpsimd.memset(ones_sb[:], 1.0)

            xk3 = xk_sb[:].rearrange("p (e t) -> p e t", e=ET)
            wk3 = wk_sb[:].rearrange("p (e t) -> p e t", e=ET)
            xq3 = xq_sb[:].rearrange("p (e t) -> p e t", e=ET)
            wq3 = wq_sb[:].rearrange("p (e t) -> p e t", e=ET)
            xv3 = xv_sb[:].rearrange("p (e t) -> p e t", e=ET)
            wv3 = wv_sb[:].rearrange("p (e t) -> p e t", e=ET)
            wo3 = wo_sb[:].rearrange("p (r t) -> p r t", r=JT)

            # ---- persistent activations ----
            qT_sb = [pp.tile([128, T], BF, name=f"qT{r}") for r in range(JT)]
            kT_sb = [pp.tile([128, SQ], BF, name=f"kT{r}") for r in range(JT)]
            v_sb = [pp.tile([128, HH * (DH + 1)], BF, name=f"v{st}") for st in range(ST)]
            aT_sb = [pp.tile([128, T], BF, name=f"aT{r}") for r in range(JT)]
            expT = [None] * (HH * ST)

            # ---- k projection (pure PE warmup while other DMAs land) ----
            for r in range(JT):
                for lo, hi in ((0, 512), (512, SQ)):
                    if lo >= SQ:
                        continue
                    ps = pps.tile([128, 512], F32, tag="aux", name="ps_k")
                    for et in range(ET):
                        nc.tensor.matmul(
                            ps[:, 0:hi - lo],
                            lhsT=wk3[:, et, r * 128:(r + 1) * 128],
                            rhs=xk3[:, et, lo:hi],
                            start=(et == 0), stop=(et == ET - 1),
                        )
                    nc.vector.tensor_scalar_add(
                        kT_sb[r][:, lo:hi], ps[:, 0:hi - lo], bk_sb[:, r:r + 1]
                    )

            # ---- unit emitters ----
            def emit_q(r):
                for lo, hi in ((0, 512), (512, 1024)):
                    ps = pps.tile([128, 512], F32, tag="aux", name="ps_q")
                    for et in range(ET):
                        nc.tensor.matmul(
                            ps[:],
                            lhsT=wq3[:, et, r * 128:(r + 1) * 128],
                            rhs=xq3[:, et, lo:hi],
                            start=(et == 0), stop=(et == ET - 1),
                        )
                    nc.vector.tensor_scalar_add(
                        qT_sb[r][:, lo:hi], ps[:], bq_sb[:, r:r + 1]
                    )

            def emit_qk(r, hl, st):
                h = 2 * r + hl
                po_ = hl * DH
                ps_s = pps.tile([128, 1024], F32, tag="qk", name="ps_s")
                for lo, hi in ((0, 512), (512, 1024)):
                    nc.tensor.matmul(
                        ps_s[:, lo:hi],
                        lhsT=kT_sb[r][po_:po_ + DH, st * 128:(st + 1) * 128],
                        rhs=qT_sb[r][po_:po_ + DH, lo:hi],
                        start=True, stop=True,
                    )
                ex = pe.tile([128, T], BF, tag="exp", name=f"exp{h}_{st}")
                expT[h * ST + st] = ex
                nc.scalar.activation(
                    ex[:], ps_s[:],
                    mybir.ActivationFunctionType.Exp,
                    bias=mb_sb[:, st:st + 1], scale=SCALING,
                )

            def emit_v(st):
                ps = pps.tile([128, 512], F32, tag="aux", name="ps_v")
                for et in range(ET):
                    nc.tensor.matmul(
                        ps[:, 0:G],
                        lhsT=xv3[:, et, st * 128:(st + 1) * 128],
                        rhs=wv3[:, et, :],
                        start=(et == 0), stop=False,
                    )
                nc.tensor.matmul(  # += ones[1,128].T @ bv[1,512]
                    ps[:, 0:G], lhsT=ones_sb[:], rhs=bv_sb[:],
                    start=False, stop=True,
                )
                v3 = v_sb[st][:].rearrange("p (h x) -> p h x", x=DH + 1)
                nc.vector.tensor_copy(
                    v3[:, :, 0:DH], ps[:, 0:G].rearrange("p (h x) -> p h x", x=DH)
                )
                nc.gpsimd.memset(v3[:, :, DH:DH + 1], 1.0)

            pav_cur = [None, None]
            op_cur = [None]

            def emit_av(hp, tt, i):
                h = 2 * hp + i
                pav = pps.tile([128, 512], F32, tag="avh", name="pav")
                pav_cur[i] = pav
                for st in range(ST):
                    nc.tensor.matmul(
                        pav[:, 0:DH + 1],
                        lhsT=expT[h * ST + st][:, tt * 128:(tt + 1) * 128],
                        rhs=v_sb[st][:, h * (DH + 1):(h + 1) * (DH + 1)],
                        start=(st == 0), stop=(st == ST - 1),
                    )
                if i == 0:
                    op_cur[0] = psm.tile([128, 128], BF, tag="op", name="op")
                op = op_cur[0]
                nc.vector.tensor_scalar(
                    op[:, i * DH:(i + 1) * DH], pav[:, 0:DH],
                    pav[:, DH:DH + 1], None, op0=mybir.AluOpType.divide,
                )
                if i == 1:
                    tr = pps.tile([128, 128], BF, tag="aux", name="tr")
                    nc.tensor.transpose(tr[:], op[:], id_sb[:])
                    nc.vector.tensor_copy(
                        aT_sb[hp][:, tt * 128:(tt + 1) * 128], tr[:]
                    )

            def emit_op(t2):
                osb = po.tile([128, E], F32, tag="osb", name="osb")
                for oc in range(2):
                    pu = pps.tile([128, 512], F32, tag="aux", name="pu")
                    for r in range(JT):
                        nc.tensor.matmul(
                            pu[:, 0:512],
                            lhsT=aT_sb[r][:, t2 * 128:(t2 + 1) * 128],
                            rhs=wo3[:, r, oc * 512:(oc + 1) * 512],
                            start=(r == 0), stop=(r == JT - 1),
                        )
                    nc.scalar.copy(osb[:, oc * 512:(oc + 1) * 512], pu[:, 0:512])
                nc.sync.dma_start(
                    out=out[t2 * 128:(t2 + 1) * 128, :], in_=osb[:]
                )

            # ---- interleaved A/B emission ----
            a_list = []
            for r in range(JT):
                a_list.append(("q", r))
                for hl in range(2):
                    for st in range(ST):
                        a_list.append(("qk", r, hl, st))
            b_list = [("v", st) for st in range(ST)]
            for hp in range(JT):
                for tt in range(TT):
                    b_list.append(("av", hp, tt, 0))
                    b_list.append(("av", hp, tt, 1))
                    if hp == JT - 1 and tt >= 1:
                        b_list.append(("op", tt - 1))
            b_list.append(("op", TT - 1))

            def emit(u):
                kind = u[0]
                if kind == "q":
                    emit_q(u[1])
                elif kind == "qk":
                    emit_qk(u[1], u[2], u[3])
                elif kind == "v":
                    emit_v(u[1])
                elif kind == "av":
                    emit_av(u[1], u[2], u[3])
                elif kind == "op":
                    emit_op(u[1])

            def a_idx(r, hl, st):
                return r * (2 * ST + 1) + 1 + hl * ST + st

            def prereq(u):
                if u[0] == "av":
                    _, hp, tt, i = u
                    return a_idx(hp, i, ST - 1)
                if u[0] == "op":
                    return a_idx(JT - 1, 1, ST - 1)
                return 0

            nA, nB = len(a_list), len(b_list)
            ib = 0
            for i, u in enumerate(a_list):
                emit(u)
                if i + 1 > off:
                    tgt = ((i + 1 - off) * nB + (nA - off) - 1) // (nA - off)
                    while ib < min(tgt, nB) and prereq(b_list[ib]) <= i:
                        emit(b_list[ib])
                        ib += 1
            while ib < nB:
                emit(b_list[ib])
                ib += 1
    return _split_waits(nc) if split else nc


_NC_CACHE = {}


def _get_nc(st_tiles=5):
    if st_tiles not in _NC_CACHE:
        _NC_CACHE[st_tiles] = build_nc(st_tiles)
    return _NC_CACHE[st_tiles]


def make_in_maps(query, key, value, key_padding_mask, local_mask,
                 Wq, bq, Wk, bk, Wv, bv, Wo, bo, st_tiles=5):
    import ml_dtypes
    f = np.float32
    bf = ml_dtypes.bfloat16
    SQ = st_tiles * 128
    ident = np.eye(128, dtype=bf)
    in_maps = []
    for c in range(8):
        b, g = c // 2, c % 2
        gs = slice(g * G, (g + 1) * G)
        mask = np.asarray((key_padding_mask if g == 0 else local_mask)[b])
        sel = np.flatnonzero(~mask)
        ns = sel.size
        assert ns <= SQ, (ns, SQ)
        xk = np.zeros((E, SQ), dtype=bf)
        xk[:, :ns] = np.asarray(key[b]).T[:, sel]
        xv = np.zeros((E, SQ), dtype=bf)
        xv[:, :ns] = np.asarray(value[b]).T[:, sel]
        mbias = np.full(SQ, NEG, f)
        mbias[:ns] = 0.0
        cfm = np.concatenate([
            np.asarray(bq)[gs].astype(f).reshape(JT, 128).T,
            np.asarray(bk)[gs].astype(f).reshape(JT, 128).T,
            mbias.reshape(st_tiles, 128).T,
        ], axis=1)
        in_maps.append({
            "xqT": np.ascontiguousarray(np.asarray(query[b]).T, dtype=bf),
            "xkT": xk,
            "xvT": xv,
            "wqT": np.ascontiguousarray(np.asarray(Wq)[gs, :].T, dtype=bf),
            "wkT": np.ascontiguousarray(np.asarray(Wk)[gs, :].T, dtype=bf),
            "wvT": np.ascontiguousarray(np.asarray(Wv)[gs, :].T, dtype=bf),
            "woT": np.ascontiguousarray(np.asarray(Wo)[:, gs].T, dtype=bf),
            "cf": np.ascontiguousarray(cfm),
            "bvr": np.ascontiguousarray(np.asarray(bv)[gs].astype(bf).reshape(1, G)),
            "idn": ident,
        })
    return in_maps


def _needed_st_tiles(key_padding_mask, local_mask):
    worst = 0
    for c in range(8):
        b, g = c // 2, c % 2
        mask = np.asarray((key_padding_mask if g == 0 else local_mask)[b])
        worst = max(worst, int((~mask).sum()))
    return max(1, -(-worst // 128))


def kernel(query, key, value, key_padding_mask, local_mask,
           Wq, bq, Wk, bk, Wv, bv, Wo, bo, _trace=False, _tmpdir=None):
    from concourse.bass_utils import run_bass_kernel_spmd

    st = min(max(_needed_st_tiles(key_padding_mask, local_mask), 5), 8)
    nc = _get_nc(st)
    in_maps = make_in_maps(query, key, value, key_padding_mask, local_mask,
                           Wq, bq, Wk, bk, Wv, bv, Wo, bo, st_tiles=st)
    try:
        res = run_bass_kernel_spmd(nc, in_maps, list(range(8)),
                                   trace=_trace, tmpdir=_tmpdir)
    except Exception:
        # transient device/transport failures have been observed on the
        # axon path; one fresh attempt is cheap relative to a hard fail
        res = run_bass_kernel_spmd(nc, in_maps, list(range(8)),
                                   trace=_trace, tmpdir=_tmpdir)
    outs = [np.asarray(r["out"]) for r in res.results]
    full = np.stack([outs[2 * b] + outs[2 * b + 1] for b in range(B)])
    full += np.asarray(bo, dtype=np.float32)
    if _trace:
        kernel._last_exec_time_ns = res.exec_time_ns
        kernel._last_profile = res.profile_json
    return full.astype(np.float32)


# revision 11
# speedup vs baseline: 1.3000x; 1.1204x over previous
"""Cross multihead attention (global/local masked head groups) on 8 trn2 cores.

Sharding: core c -> (batch b = c//2, head-group g = c%2).
  g=0: heads 0-7  masked by key_padding_mask[b]
  g=1: heads 8-15 masked by local_mask[b]
Each core computes its group's partial output (attn_out_g @ Wo[:, gs].T)
of shape [T, E]; the host sums the two partials per batch and adds bo.

Key structure:
  - Masked keys contribute nothing (exp -> 0), so the host gathers the
    ~50% unmasked key/value columns into an SQ=640-padded buffer;
    k/v projections, QK^T and AV all shrink accordingly.  Padding
    columns keep a -30000 exp bias so they vanish from the softmax.
  - scoresT orientation [s, t]: mask bias is per-partition for the exp.
  - AV is computed as out[t, d] = expT.T @ [v | ones]: the moving operand
    is only 65 wide (PE cost ~ moving free size), and the ones column
    gives the softmax denominator per t-partition, so normalization is
    a per-partition scalar multiply (no partition broadcast needed).
  - A PE transpose (identity moving operand) flips normalized [t, j]
    head pairs back to [j, t] for the output projection.
  - All big DRAM loads are single batched DMAs (HWDGE serializes at
    ~625ns per dma_start).
  - Software-pipelined emission: the PE stream interleaves "A units"
    (q-proj + QK + exp, which pace at the Act engine's exp speed via
    PSUM buffer rotation) with "B units" (v-proj, AV chains, output
    projection) so the PE never idles while Act catches up.
"""

import os
import sys

sys.path.insert(0, "/opt/trn_rl_repo")

import numpy as np

import concourse.bass as bass
import concourse.mybir as mybir
from concourse.tile import TileContext

B, T, S, E, H = 4, 1024, 1024, 1024, 16
DH = E // H            # 64
HH = H // 2            # 8 heads per group
G = HH * DH            # 512 features per group
SCALING = DH ** -0.5
NEG = -30000.0         # exp(x + NEG) == 0.0 in fp32, no LUT edge cases

F32 = mybir.dt.float32
BF = mybir.dt.bfloat16

ET = E // 128          # 8 contraction tiles
JT = G // 128          # 4 j-tiles (head pairs)
TT = T // 128          # 8 t tiles


def _split_waits(nc):
    """TPB ISA structs hold one sem-wait slot. Tile can emit >1 wait per
    instruction (walrus: 'Too many sync wait commands'); hoist all but the
    last wait onto single-wait NOPs on the same engine, inserted just
    before. Timing is unchanged - the waits would have blocked anyway."""
    k = 0
    for f in nc.m.functions:
        for blk in f.blocks:
            new = []
            for inst in blk.instructions:
                si = inst.sync_info
                w = list(si.on_wait) if si else []
                if len(w) > 1:
                    for wait in w[:-1]:
                        nop = mybir.InstNoOp(name=f"nopw-{k}", ins=[], outs=[])
                        k += 1
                        nop.engine = inst.engine
                        nop.sync_info = mybir.SyncInfo(on_wait=[wait], on_update=[])
                        new.append(nop)
                    inst.sync_info = mybir.SyncInfo(
                        on_wait=[w[-1]], on_update=list(si.on_update)
                    )
                new.append(inst)
            blk.instructions = new
    return nc


def build_nc(st_tiles=5, split=True, off=11):
    ST = st_tiles          # number of 128-wide s tiles after compression
    SQ = ST * 128          # padded compressed key count

    nc = bass.Bass()

    xqT = nc.dram_tensor("xqT", [E, T], BF, kind="ExternalInput")
    xkT = nc.dram_tensor("xkT", [E, SQ], BF, kind="ExternalInput")
    xvT = nc.dram_tensor("xvT", [E, SQ], BF, kind="ExternalInput")
    wqT = nc.dram_tensor("wqT", [E, G], BF, kind="ExternalInput")
    wkT = nc.dram_tensor("wkT", [E, G], BF, kind="ExternalInput")
    wvT = nc.dram_tensor("wvT", [E, G], BF, kind="ExternalInput")
    woT = nc.dram_tensor("woT", [G, E], BF, kind="ExternalInput")
    # packed f32 consts: bq | bk | mb  -> [128, 4 + 4 + ST]
    cf = nc.dram_tensor("cf", [128, 2 * JT + ST], F32, kind="ExternalInput")
    bvr = nc.dram_tensor("bvr", [1, G], BF, kind="ExternalInput")
    idn = nc.dram_tensor("idn", [128, 128], BF, kind="ExternalInput")
    out = nc.dram_tensor("out", [T, E], F32, kind="ExternalOutput")

    with TileContext(nc) as tc:
        with (
            tc.tile_pool(name="const", bufs=1) as pc,
            tc.tile_pool(name="persist", bufs=1) as pp,
            tc.tile_pool(name="exp", bufs=HH * ST) as pe,
            tc.tile_pool(name="small", bufs=4) as psm,
            tc.tile_pool(name="outsb", bufs=2) as po,
            tc.tile_pool(name="ps", bufs=2, space="PSUM") as pps,
        ):
            # ---- staged loads, startup-latency ordered ----
            # wk/xk land in halves so the k projection starts ~5us earlier
            wk_sb = pc.tile([128, ET * G], BF, name="wk_sb")
            xk_sb = pc.tile([128, ET * SQ], BF, name="xk_sb")
            EH = ET // 2
            for eh in range(2):
                nc.sync.dma_start(
                    out=wk_sb[:].rearrange("p (e t) -> p e t", e=ET)[:, eh * EH:(eh + 1) * EH],
                    in_=wkT[:].rearrange("(e p) t -> p e t", p=128)[:, eh * EH:(eh + 1) * EH],
                )
                nc.sync.dma_start(
                    out=xk_sb[:].rearrange("p (e t) -> p e t", e=ET)[:, eh * EH:(eh + 1) * EH],
                    in_=xkT[:].rearrange("(e p) t -> p e t", p=128)[:, eh * EH:(eh + 1) * EH],
                )
            cf_sb = pc.tile([128, 2 * JT + ST], F32, name="cf_sb")
            nc.sync.dma_start(out=cf_sb[:], in_=cf[:])
            bq_sb = cf_sb[:, 0:JT]
            bk_sb = cf_sb[:, JT:2 * JT]
            mb_sb = cf_sb[:, 2 * JT:]

            wq_sb = pc.tile([128, ET * G], BF, name="wq_sb")
            nc.sync.dma_start(
                out=wq_sb[:].rearrange("p (e t) -> p e t", e=ET),
                in_=wqT[:].rearrange("(e p) t -> p e t", p=128),
            )
            xq_sb = pc.tile([128, ET * T], BF, name="xq_sb")
            nc.sync.dma_start(
                out=xq_sb[:].rearrange("p (e t) -> p e t", e=ET),
                in_=xqT[:].rearrange("(e p) t -> p e t", p=128),
            )
            bv_sb = pc.tile([1, G], BF, name="bv_sb")
            nc.sync.dma_start(out=bv_sb[:], in_=bvr[:])
            id_sb = pc.tile([128, 128], BF, name="id_sb")
            nc.sync.dma_start(out=id_sb[:], in_=idn[:])
            wv_sb = pc.tile([128, ET * G], BF, name="wv_sb")
            nc.sync.dma_start(
                out=wv_sb[:].rearrange("p (e t) -> p e t", e=ET),
                in_=wvT[:].rearrange("(e p) t -> p e t", p=128),
            )
            xv_sb = pc.tile([128, ET * SQ], BF, name="xv_sb")
            nc.sync.dma_start(
                out=xv_sb[:].rearrange("p (e t) -> p e t", e=ET),
                in_=xvT[:].rearrange("(e p) t -> p e t", p=128),
            )
            wo_sb = pc.tile([128, JT * E], BF, name="wo_sb")
            nc.sync.dma_start(
                out=wo_sb[:].rearrange("p (r t) -> p r t", r=JT),
                in_=woT[:].rearrange("(r p) t -> p r t", p=128),
            )
            ones_sb = pc.tile([1, 128], BF, name="ones_sb")
            nc.gpsimd.memset(ones_sb[:], 1.0)

            xk3 = xk_sb[:].rearrange("p (e t) -> p e t", e=ET)
            wk3 = wk_sb[:].rearrange("p (e t) -> p e t", e=ET)
            xq3 = xq_sb[:].rearrange("p (e t) -> p e t", e=ET)
            wq3 = wq_sb[:].rearrange("p (e t) -> p e t", e=ET)
            xv3 = xv_sb[:].rearrange("p (e t) -> p e t", e=ET)
            wv3 = wv_sb[:].rearrange("p (e t) -> p e t", e=ET)
            wo3 = wo_sb[:].rearrange("p (r t) -> p r t", r=JT)

            # ---- persistent activations ----
            qT_sb = [pp.tile([128, T], BF, name=f"qT{r}") for r in range(JT)]
            kT_sb = [pp.tile([128, SQ], BF, name=f"kT{r}") for r in range(JT)]
            v_sb = [pp.tile([128, HH * (DH + 1)], BF, name=f"v{st}") for st in range(ST)]
            aT_sb = [pp.tile([128, T], BF, name=f"aT{r}") for r in range(JT)]
            expT = [None] * (HH * ST)

            # ---- k projection (pure PE warmup while other DMAs land) ----
            for r in range(JT):
                for lo, hi in ((0, 512), (512, SQ)):
                    if lo >= SQ:
                        continue
                    ps = pps.tile([128, 512], F32, tag="aux", name="ps_k")
                    for et in range(ET):
                        nc.tensor.matmul(
                            ps[:, 0:hi - lo],
                            lhsT=wk3[:, et, r * 128:(r + 1) * 128],
                            rhs=xk3[:, et, lo:hi],
                            start=(et == 0), stop=(et == ET - 1),
                        )
                    nc.vector.tensor_scalar_add(
                        kT_sb[r][:, lo:hi], ps[:, 0:hi - lo], bk_sb[:, r:r + 1]
                    )

            # ---- unit emitters ----
            def emit_q(r):
                for lo, hi in ((0, 512), (512, 1024)):
                    ps = pps.tile([128, 512], F32, tag="aux", name="ps_q")
                    for et in range(ET):
                        nc.tensor.matmul(
                            ps[:],
                            lhsT=wq3[:, et, r * 128:(r + 1) * 128],
                            rhs=xq3[:, et, lo:hi],
                            start=(et == 0), stop=(et == ET - 1),
                        )
                    nc.vector.tensor_scalar_add(
                        qT_sb[r][:, lo:hi], ps[:], bq_sb[:, r:r + 1]
                    )

            def emit_qk(r, hl, st):
                h = 2 * r + hl
                po_ = hl * DH
                ps_s = pps.tile([128, 1024], F32, tag="qk", name="ps_s")
                for lo, hi in ((0, 512), (512, 1024)):
                    nc.tensor.matmul(
                        ps_s[:, lo:hi],
                        lhsT=kT_sb[r][po_:po_ + DH, st * 128:(st + 1) * 128],
                        rhs=qT_sb[r][po_:po_ + DH, lo:hi],
                        start=True, stop=True,
                    )
                ex = pe.tile([128, T], BF, tag="exp", name=f"exp{h}_{st}")
                expT[h * ST + st] = ex
                nc.scalar.activation(
                    ex[:], ps_s[:],
                    mybir.ActivationFunctionType.Exp,
                    bias=mb_sb[:, st:st + 1], scale=SCALING,
                )

            def emit_v(st):
                ps = pps.tile([128, 512], F32, tag="aux", name="ps_v")
                for et in range(ET):
                    nc.tensor.matmul(
                        ps[:, 0:G],
                        lhsT=xv3[:, et, st * 128:(st + 1) * 128],
                        rhs=wv3[:, et, :],
                        start=(et == 0), stop=False,
                    )
                nc.tensor.matmul(  # += ones[1,128].T @ bv[1,512]
                    ps[:, 0:G], lhsT=ones_sb[:], rhs=bv_sb[:],
                    start=False, stop=True,
                )
                v3 = v_sb[st][:].rearrange("p (h x) -> p h x", x=DH + 1)
                nc.vector.tensor_copy(
                    v3[:, :, 0:DH], ps[:, 0:G].rearrange("p (h x) -> p h x", x=DH)
                )
                nc.gpsimd.memset(v3[:, :, DH:DH + 1], 1.0)

            pav_cur = [None, None]
            op_cur = [None]

            def emit_av(hp, tt, i):
                h = 2 * hp + i
                pav = pps.tile([128, 512], F32, tag="avh", name="pav")
                pav_cur[i] = pav
                for st in range(ST):
                    nc.tensor.matmul(
                        pav[:, 0:DH + 1],
                        lhsT=expT[h * ST + st][:, tt * 128:(tt + 1) * 128],
                        rhs=v_sb[st][:, h * (DH + 1):(h + 1) * (DH + 1)],
                        start=(st == 0), stop=(st == ST - 1),
                    )
                if i == 0:
                    op_cur[0] = psm.tile([128, 128], BF, tag="op", name="op")
                op = op_cur[0]
                nc.vector.tensor_scalar(
                    op[:, i * DH:(i + 1) * DH], pav[:, 0:DH],
                    pav[:, DH:DH + 1], None, op0=mybir.AluOpType.divide,
                )
                if i == 1:
                    tr = pps.tile([128, 128], BF, tag="aux", name="tr")
                    nc.tensor.transpose(tr[:], op[:], id_sb[:])
                    nc.vector.tensor_copy(
                        aT_sb[hp][:, tt * 128:(tt + 1) * 128], tr[:]
                    )

            def emit_op(t2):
                osb = po.tile([128, E], F32, tag="osb", name="osb")
                for oc in range(2):
                    pu = pps.tile([128, 512], F32, tag="aux", name="pu")
                    for r in range(JT):
                        nc.tensor.matmul(
                            pu[:, 0:512],
                            lhsT=aT_sb[r][:, t2 * 128:(t2 + 1) * 128],
                            rhs=wo3[:, r, oc * 512:(oc + 1) * 512],
                            start=(r == 0), stop=(r == JT - 1),
                        )
                    if oc == 0:
                        nc.vector.tensor_copy(osb[:, 0:512], pu[:, 0:512])
                    else:
                        nc.scalar.copy(osb[:, 512:1024], pu[:, 0:512])
                    nc.sync.dma_start(
                        out=out[t2 * 128:(t2 + 1) * 128,
                                oc * 512:(oc + 1) * 512],
                        in_=osb[:, oc * 512:(oc + 1) * 512],
                    )

            # ---- interleaved A/B emission ----
            a_list = []
            for r in range(JT):
                a_list.append(("q", r))
                for hl in range(2):
                    for st in range(ST):
                        a_list.append(("qk", r, hl, st))
            b_list = [("v", st) for st in range(ST)]
            for hp in range(JT):
                for tt in range(TT):
                    b_list.append(("av", hp, tt, 0))
                    b_list.append(("av", hp, tt, 1))
                    if hp == JT - 1 and tt >= 1:
                        b_list.append(("op", tt - 1))
            b_list.append(("op", TT - 1))

            def emit(u):
                kind = u[0]
                if kind == "q":
                    emit_q(u[1])
                elif kind == "qk":
                    emit_qk(u[1], u[2], u[3])
                elif kind == "v":
                    emit_v(u[1])
                elif kind == "av":
                    emit_av(u[1], u[2], u[3])
                elif kind == "op":
                    emit_op(u[1])

            def a_idx(r, hl, st):
                return r * (2 * ST + 1) + 1 + hl * ST + st

            ua = 2 * ST + 1   # A-units per j-tile

            def min_a(u):
                if u[0] == "v":
                    return off + 1
                if u[0] == "av":
                    _, hp, tt, i = u
                    spread = off + 1 + ua * hp + ((tt * 2 + i) * ua) // 16
                    return max(a_idx(hp, i, ST - 1), spread)
                if u[0] == "op":
                    return a_idx(JT - 1, 1, ST - 1)
                return 0

            nA, nB = len(a_list), len(b_list)
            ib = 0
            for i, u in enumerate(a_list):
                emit(u)
                while ib < nB and min_a(b_list[ib]) <= i:
                    emit(b_list[ib])
                    ib += 1
            while ib < nB:
                emit(b_list[ib])
                ib += 1
    return _split_waits(nc) if split else nc


_NC_CACHE = {}


def _get_nc(st_tiles=5):
    if st_tiles not in _NC_CACHE:
        _NC_CACHE[st_tiles] = build_nc(st_tiles)
    return _NC_CACHE[st_tiles]


def make_in_maps(query, key, value, key_padding_mask, local_mask,
                 Wq, bq, Wk, bk, Wv, bv, Wo, bo, st_tiles=5):
    import ml_dtypes
    f = np.float32
    bf = ml_dtypes.bfloat16
    SQ = st_tiles * 128
    ident = np.eye(128, dtype=bf)
    in_maps = []
    for c in range(8):
        b, g = c // 2, c % 2
        gs = slice(g * G, (g + 1) * G)
        mask = np.asarray((key_padding_mask if g == 0 else local_mask)[b])
        sel = np.flatnonzero(~mask)
        ns = sel.size
        assert ns <= SQ, (ns, SQ)
        xk = np.zeros((E, SQ), dtype=bf)
        xk[:, :ns] = np.asarray(key[b]).T[:, sel]
        xv = np.zeros((E, SQ), dtype=bf)
        xv[:, :ns] = np.asarray(value[b]).T[:, sel]
        mbias = np.full(SQ, NEG, f)
        mbias[:ns] = 0.0
        cfm = np.concatenate([
            np.asarray(bq)[gs].astype(f).reshape(JT, 128).T,
            np.asarray(bk)[gs].astype(f).reshape(JT, 128).T,
            mbias.reshape(st_tiles, 128).T,
        ], axis=1)
        in_maps.append({
            "xqT": np.ascontiguousarray(np.asarray(query[b]).T, dtype=bf),
            "xkT": xk,
            "xvT": xv,
            "wqT": np.ascontiguousarray(np.asarray(Wq)[gs, :].T, dtype=bf),
            "wkT": np.ascontiguousarray(np.asarray(Wk)[gs, :].T, dtype=bf),
            "wvT": np.ascontiguousarray(np.asarray(Wv)[gs, :].T, dtype=bf),
            "woT": np.ascontiguousarray(np.asarray(Wo)[:, gs].T, dtype=bf),
            "cf": np.ascontiguousarray(cfm),
            "bvr": np.ascontiguousarray(np.asarray(bv)[gs].astype(bf).reshape(1, G)),
            "idn": ident,
        })
    return in_maps


def _needed_st_tiles(key_padding_mask, local_mask):
    worst = 0
    for c in range(8):
        b, g = c // 2, c % 2
        mask = np.asarray((key_padding_mask if g == 0 else local_mask)[b])
        worst = max(worst, int((~mask).sum()))
    return max(1, -(-worst // 128))


def kernel(query, key, value, key_padding_mask, local_mask,
           Wq, bq, Wk, bk, Wv, bv, Wo, bo, _trace=False, _tmpdir=None):
    from concourse.bass_utils import run_bass_kernel_spmd

    st = min(max(_needed_st_tiles(key_padding_mask, local_mask), 5), 8)
    nc = _get_nc(st)
    in_maps = make_in_maps(query, key, value, key_padding_mask, local_mask,
                           Wq, bq, Wk, bk, Wv, bv, Wo, bo, st_tiles=st)
    try:
        res = run_bass_kernel_spmd(nc, in_maps, list(range(8)),
                                   trace=_trace, tmpdir=_tmpdir)
    except Exception:
        # transient device/transport failures have been observed on the
        # axon path; one fresh attempt is cheap relative to a hard fail
        res = run_bass_kernel_spmd(nc, in_maps, list(range(8)),
                                   trace=_trace, tmpdir=_tmpdir)
    outs = [np.asarray(r["out"]) for r in res.results]
    full = np.stack([outs[2 * b] + outs[2 * b + 1] for b in range(B)])
    full += np.asarray(bo, dtype=np.float32)
    if _trace:
        kernel._last_exec_time_ns = res.exec_time_ns
        kernel._last_profile = res.profile_json
    return full.astype(np.float32)


# revision 12
# speedup vs baseline: 1.3084x; 1.0065x over previous
"""Cross multihead attention (global/local masked head groups) on 8 trn2 cores.

Sharding: core c -> (batch b = c//2, head-group g = c%2).
  g=0: heads 0-7  masked by key_padding_mask[b]
  g=1: heads 8-15 masked by local_mask[b]
Each core computes its group's partial output (attn_out_g @ Wo[:, gs].T)
of shape [T, E]; the host sums the two partials per batch and adds bo.

Key structure:
  - Masked keys contribute nothing (exp -> 0), so the host gathers the
    ~50% unmasked key/value columns into an SQ=640-padded buffer;
    k/v projections, QK^T and AV all shrink accordingly.  Padding
    columns keep a -30000 exp bias so they vanish from the softmax.
  - scoresT orientation [s, t]: mask bias is per-partition for the exp.
  - AV is computed as out[t, d] = expT.T @ [v | ones]: the moving operand
    is only 65 wide (PE cost ~ moving free size), and the ones column
    gives the softmax denominator per t-partition, so normalization is
    a per-partition scalar multiply (no partition broadcast needed).
  - A PE transpose (identity moving operand) flips normalized [t, j]
    head pairs back to [j, t] for the output projection.
  - All big DRAM loads are single batched DMAs (HWDGE serializes at
    ~625ns per dma_start).
  - Software-pipelined emission: the PE stream interleaves "A units"
    (q-proj + QK + exp, which pace at the Act engine's exp speed via
    PSUM buffer rotation) with "B units" (v-proj, AV chains, output
    projection) so the PE never idles while Act catches up.
"""

import os
import sys

sys.path.insert(0, "/opt/trn_rl_repo")

import numpy as np

import concourse.bass as bass
import concourse.mybir as mybir
from concourse.tile import TileContext

B, T, S, E, H = 4, 1024, 1024, 1024, 16
DH = E // H            # 64
HH = H // 2            # 8 heads per group
G = HH * DH            # 512 features per group
SCALING = DH ** -0.5
NEG = -30000.0         # exp(x + NEG) == 0.0 in fp32, no LUT edge cases

F32 = mybir.dt.float32
BF = mybir.dt.bfloat16

ET = E // 128          # 8 contraction tiles
JT = G // 128          # 4 j-tiles (head pairs)
TT = T // 128          # 8 t tiles


def _split_waits(nc):
    """TPB ISA structs hold one sem-wait slot. Tile can emit >1 wait per
    instruction (walrus: 'Too many sync wait commands'); hoist all but the
    last wait onto single-wait NOPs on the same engine, inserted just
    before. Timing is unchanged - the waits would have blocked anyway."""
    k = 0
    for f in nc.m.functions:
        for blk in f.blocks:
            new = []
            for inst in blk.instructions:
                si = inst.sync_info
                w = list(si.on_wait) if si else []
                if len(w) > 1:
                    for wait in w[:-1]:
                        nop = mybir.InstNoOp(name=f"nopw-{k}", ins=[], outs=[])
                        k += 1
                        nop.engine = inst.engine
                        nop.sync_info = mybir.SyncInfo(on_wait=[wait], on_update=[])
                        new.append(nop)
                    inst.sync_info = mybir.SyncInfo(
                        on_wait=[w[-1]], on_update=list(si.on_update)
                    )
                new.append(inst)
            blk.instructions = new
    return nc


def build_nc(st_tiles=5, split=True, off=11):
    ST = st_tiles          # number of 128-wide s tiles after compression
    SQ = ST * 128          # padded compressed key count

    nc = bass.Bass()

    xqT = nc.dram_tensor("xqT", [E, T], BF, kind="ExternalInput")
    xkT = nc.dram_tensor("xkT", [E, SQ], BF, kind="ExternalInput")
    xvT = nc.dram_tensor("xvT", [E, SQ], BF, kind="ExternalInput")
    wqT = nc.dram_tensor("wqT", [E, G], BF, kind="ExternalInput")
    wkT = nc.dram_tensor("wkT", [E, G], BF, kind="ExternalInput")
    wvT = nc.dram_tensor("wvT", [E, G], BF, kind="ExternalInput")
    woT = nc.dram_tensor("woT", [G, E], BF, kind="ExternalInput")
    # packed f32 consts: bq | bk | mb  -> [128, 4 + 4 + ST]
    cf = nc.dram_tensor("cf", [128, 2 * JT + ST], F32, kind="ExternalInput")
    bvr = nc.dram_tensor("bvr", [1, G], BF, kind="ExternalInput")
    idn = nc.dram_tensor("idn", [128, 128], BF, kind="ExternalInput")
    out = nc.dram_tensor("out", [T, E], F32, kind="ExternalOutput")

    with TileContext(nc) as tc:
        with (
            tc.tile_pool(name="const", bufs=1) as pc,
            tc.tile_pool(name="persist", bufs=1) as pp,
            tc.tile_pool(name="exp", bufs=HH * ST) as pe,
            tc.tile_pool(name="small", bufs=4) as psm,
            tc.tile_pool(name="outsb", bufs=2) as po,
            tc.tile_pool(name="ps", bufs=2, space="PSUM") as pps,
        ):
            # ---- staged loads, startup-latency ordered ----
            # wk/xk land in quarters so the k projection starts ~3us in
            wk_sb = pc.tile([128, ET * G], BF, name="wk_sb")
            xk_sb = pc.tile([128, ET * SQ], BF, name="xk_sb")
            EH = ET // 4
            for eh in range(4):
                nc.sync.dma_start(
                    out=wk_sb[:].rearrange("p (e t) -> p e t", e=ET)[:, eh * EH:(eh + 1) * EH],
                    in_=wkT[:].rearrange("(e p) t -> p e t", p=128)[:, eh * EH:(eh + 1) * EH],
                )
                nc.sync.dma_start(
                    out=xk_sb[:].rearrange("p (e t) -> p e t", e=ET)[:, eh * EH:(eh + 1) * EH],
                    in_=xkT[:].rearrange("(e p) t -> p e t", p=128)[:, eh * EH:(eh + 1) * EH],
                )
            cf_sb = pc.tile([128, 2 * JT + ST], F32, name="cf_sb")
            nc.sync.dma_start(out=cf_sb[:], in_=cf[:])
            bq_sb = cf_sb[:, 0:JT]
            bk_sb = cf_sb[:, JT:2 * JT]
            mb_sb = cf_sb[:, 2 * JT:]

            wq_sb = pc.tile([128, ET * G], BF, name="wq_sb")
            nc.sync.dma_start(
                out=wq_sb[:].rearrange("p (e t) -> p e t", e=ET),
                in_=wqT[:].rearrange("(e p) t -> p e t", p=128),
            )
            xq_sb = pc.tile([128, ET * T], BF, name="xq_sb")
            nc.sync.dma_start(
                out=xq_sb[:].rearrange("p (e t) -> p e t", e=ET),
                in_=xqT[:].rearrange("(e p) t -> p e t", p=128),
            )
            bv_sb = pc.tile([1, G], BF, name="bv_sb")
            nc.sync.dma_start(out=bv_sb[:], in_=bvr[:])
            id_sb = pc.tile([128, 128], BF, name="id_sb")
            nc.sync.dma_start(out=id_sb[:], in_=idn[:])
            wv_sb = pc.tile([128, ET * G], BF, name="wv_sb")
            nc.sync.dma_start(
                out=wv_sb[:].rearrange("p (e t) -> p e t", e=ET),
                in_=wvT[:].rearrange("(e p) t -> p e t", p=128),
            )
            xv_sb = pc.tile([128, ET * SQ], BF, name="xv_sb")
            nc.sync.dma_start(
                out=xv_sb[:].rearrange("p (e t) -> p e t", e=ET),
                in_=xvT[:].rearrange("(e p) t -> p e t", p=128),
            )
            wo_sb = pc.tile([128, JT * E], BF, name="wo_sb")
            nc.sync.dma_start(
                out=wo_sb[:].rearrange("p (r t) -> p r t", r=JT),
                in_=woT[:].rearrange("(r p) t -> p r t", p=128),
            )
            ones_sb = pc.tile([1, 128], BF, name="ones_sb")
            nc.gpsimd.memset(ones_sb[:], 1.0)

            xk3 = xk_sb[:].rearrange("p (e t) -> p e t", e=ET)
            wk3 = wk_sb[:].rearrange("p (e t) -> p e t", e=ET)
            xq3 = xq_sb[:].rearrange("p (e t) -> p e t", e=ET)
            wq3 = wq_sb[:].rearrange("p (e t) -> p e t", e=ET)
            xv3 = xv_sb[:].rearrange("p (e t) -> p e t", e=ET)
            wv3 = wv_sb[:].rearrange("p (e t) -> p e t", e=ET)
            wo3 = wo_sb[:].rearrange("p (r t) -> p r t", r=JT)

            # ---- persistent activations ----
            qT_sb = [pp.tile([128, T], BF, name=f"qT{r}") for r in range(JT)]
            kT_sb = [pp.tile([128, SQ], BF, name=f"kT{r}") for r in range(JT)]
            v_sb = [pp.tile([128, HH * (DH + 1)], BF, name=f"v{st}") for st in range(ST)]
            aT_sb = [pp.tile([128, T], BF, name=f"aT{r}") for r in range(JT)]
            expT = [None] * (HH * ST)

            # ---- k projection (pure PE warmup while other DMAs land) ----
            for r in range(JT):
                for lo, hi in ((0, 512), (512, SQ)):
                    if lo >= SQ:
                        continue
                    ps = pps.tile([128, 512], F32, tag="aux", name="ps_k")
                    for et in range(ET):
                        nc.tensor.matmul(
                            ps[:, 0:hi - lo],
                            lhsT=wk3[:, et, r * 128:(r + 1) * 128],
                            rhs=xk3[:, et, lo:hi],
                            start=(et == 0), stop=(et == ET - 1),
                        )
                    nc.vector.tensor_scalar_add(
                        kT_sb[r][:, lo:hi], ps[:, 0:hi - lo], bk_sb[:, r:r + 1]
                    )

            # ---- unit emitters ----
            def emit_q(r):
                for lo, hi in ((0, 512), (512, 1024)):
                    ps = pps.tile([128, 512], F32, tag="aux", name="ps_q")
                    for et in range(ET):
                        nc.tensor.matmul(
                            ps[:],
                            lhsT=wq3[:, et, r * 128:(r + 1) * 128],
                            rhs=xq3[:, et, lo:hi],
                            start=(et == 0), stop=(et == ET - 1),
                        )
                    nc.vector.tensor_scalar_add(
                        qT_sb[r][:, lo:hi], ps[:], bq_sb[:, r:r + 1]
                    )

            def emit_qk(r, hl, st):
                h = 2 * r + hl
                po_ = hl * DH
                ps_s = pps.tile([128, 1024], F32, tag="qk", name="ps_s")
                for lo, hi in ((0, 512), (512, 1024)):
                    nc.tensor.matmul(
                        ps_s[:, lo:hi],
                        lhsT=kT_sb[r][po_:po_ + DH, st * 128:(st + 1) * 128],
                        rhs=qT_sb[r][po_:po_ + DH, lo:hi],
                        start=True, stop=True,
                    )
                ex = pe.tile([128, T], BF, tag="exp", name=f"exp{h}_{st}")
                expT[h * ST + st] = ex
                nc.scalar.activation(
                    ex[:], ps_s[:],
                    mybir.ActivationFunctionType.Exp,
                    bias=mb_sb[:, st:st + 1], scale=SCALING,
                )

            bvb_sb = pc.tile([128, G], BF, name="bvb_sb")

            def emit_v(st):
                if st == 0:
                    psb = pps.tile([128, 512], F32, tag="aux", name="ps_bvb")
                    nc.tensor.matmul(  # bv broadcast: ones[1,128].T @ bv[1,512]
                        psb[:, 0:G], lhsT=ones_sb[:], rhs=bv_sb[:],
                        start=True, stop=True,
                    )
                    nc.vector.tensor_copy(bvb_sb[:], psb[:, 0:G])
                ps = pps.tile([128, 512], F32, tag="aux", name="ps_v")
                for et in range(ET):
                    nc.tensor.matmul(
                        ps[:, 0:G],
                        lhsT=xv3[:, et, st * 128:(st + 1) * 128],
                        rhs=wv3[:, et, :],
                        start=(et == 0), stop=(et == ET - 1),
                    )
                v3 = v_sb[st][:].rearrange("p (h x) -> p h x", x=DH + 1)
                nc.vector.tensor_tensor(
                    v3[:, :, 0:DH],
                    ps[:, 0:G].rearrange("p (h x) -> p h x", x=DH),
                    bvb_sb[:].rearrange("p (h x) -> p h x", x=DH),
                    op=mybir.AluOpType.add,
                )
                nc.gpsimd.memset(v3[:, :, DH:DH + 1], 1.0)

            pav_cur = [None, None]
            op_cur = [None]

            def emit_av(hp, tt, i, defer_tr=False):
                h = 2 * hp + i
                pav = pps.tile([128, 512], F32, tag="avh", name="pav")
                pav_cur[i] = pav
                for st in range(ST):
                    nc.tensor.matmul(
                        pav[:, 0:DH + 1],
                        lhsT=expT[h * ST + st][:, tt * 128:(tt + 1) * 128],
                        rhs=v_sb[st][:, h * (DH + 1):(h + 1) * (DH + 1)],
                        start=(st == 0), stop=(st == ST - 1),
                    )
                if i == 0:
                    op_cur[0] = psm.tile([128, 128], BF, tag="op", name="op")
                op = op_cur[0]
                nc.vector.tensor_scalar(
                    op[:, i * DH:(i + 1) * DH], pav[:, 0:DH],
                    pav[:, DH:DH + 1], None, op0=mybir.AluOpType.divide,
                )
                if i == 1 and not defer_tr:
                    emit_trc(hp, tt)

            def emit_trc(hp, tt):
                op = op_cur[0]
                tr = pps.tile([128, 128], BF, tag="aux", name="tr")
                nc.tensor.transpose(tr[:], op[:], id_sb[:])
                nc.vector.tensor_copy(
                    aT_sb[hp][:, tt * 128:(tt + 1) * 128], tr[:]
                )

            def emit_op(t2):
                osb = po.tile([128, E], F32, tag="osb", name="osb")
                for oc in range(2):
                    pu = pps.tile([128, 512], F32, tag="aux", name="pu")
                    for r in range(JT):
                        nc.tensor.matmul(
                            pu[:, 0:512],
                            lhsT=aT_sb[r][:, t2 * 128:(t2 + 1) * 128],
                            rhs=wo3[:, r, oc * 512:(oc + 1) * 512],
                            start=(r == 0), stop=(r == JT - 1),
                        )
                    if oc == 0:
                        nc.vector.tensor_copy(osb[:, 0:512], pu[:, 0:512])
                    else:
                        nc.scalar.copy(osb[:, 512:1024], pu[:, 0:512])
                    nc.sync.dma_start(
                        out=out[t2 * 128:(t2 + 1) * 128,
                                oc * 512:(oc + 1) * 512],
                        in_=osb[:, oc * 512:(oc + 1) * 512],
                    )

            # ---- interleaved A/B emission ----
            a_list = []
            for r in range(JT):
                a_list.append(("q", r))
                for hl in range(2):
                    for st in range(ST):
                        a_list.append(("qk", r, hl, st))
            b_list = [("v", st) for st in range(ST)]
            for hp in range(JT):
                for tt in range(TT):
                    b_list.append(("av", hp, tt, 0))
                    if hp == JT - 1:
                        b_list.append(("avd", hp, tt, 1))
                        if tt >= 1:
                            b_list.append(("op", tt - 1))
                        b_list.append(("trc", hp, tt))
                    else:
                        b_list.append(("av", hp, tt, 1))
            b_list.append(("op", TT - 1))

            def emit(u):
                kind = u[0]
                if kind == "q":
                    emit_q(u[1])
                elif kind == "qk":
                    emit_qk(u[1], u[2], u[3])
                elif kind == "v":
                    emit_v(u[1])
                elif kind == "av":
                    emit_av(u[1], u[2], u[3])
                elif kind == "avd":
                    emit_av(u[1], u[2], u[3], defer_tr=True)
                elif kind == "trc":
                    emit_trc(u[1], u[2])
                elif kind == "op":
                    emit_op(u[1])

            def a_idx(r, hl, st):
                return r * (2 * ST + 1) + 1 + hl * ST + st

            ua = 2 * ST + 1   # A-units per j-tile

            def min_a(u):
                if u[0] == "v":
                    return off + 1
                if u[0] in ("av", "avd"):
                    _, hp, tt, i = u
                    spread = off + 1 + ua * hp + ((tt * 2 + i) * ua) // 16
                    return max(a_idx(hp, i, ST - 1), spread)
                if u[0] == "trc":
                    return a_idx(u[1], 1, ST - 1)
                if u[0] == "op":
                    return a_idx(JT - 1, 1, ST - 1)
                return 0

            nA, nB = len(a_list), len(b_list)
            ib = 0
            for i, u in enumerate(a_list):
                emit(u)
                while ib < nB and min_a(b_list[ib]) <= i:
                    emit(b_list[ib])
                    ib += 1
            while ib < nB:
                emit(b_list[ib])
                ib += 1
    return _split_waits(nc) if split else nc


_NC_CACHE = {}


def _get_nc(st_tiles=5):
    if st_tiles not in _NC_CACHE:
        _NC_CACHE[st_tiles] = build_nc(st_tiles)
    return _NC_CACHE[st_tiles]


def make_in_maps(query, key, value, key_padding_mask, local_mask,
                 Wq, bq, Wk, bk, Wv, bv, Wo, bo, st_tiles=5):
    import ml_dtypes
    f = np.float32
    bf = ml_dtypes.bfloat16
    SQ = st_tiles * 128
    ident = np.eye(128, dtype=bf)
    in_maps = []
    for c in range(8):
        b, g = c // 2, c % 2
        gs = slice(g * G, (g + 1) * G)
        mask = np.asarray((key_padding_mask if g == 0 else local_mask)[b])
        sel = np.flatnonzero(~mask)
        ns = sel.size
        assert ns <= SQ, (ns, SQ)
        xk = np.zeros((E, SQ), dtype=bf)
        xk[:, :ns] = np.asarray(key[b]).T[:, sel]
        xv = np.zeros((E, SQ), dtype=bf)
        xv[:, :ns] = np.asarray(value[b]).T[:, sel]
        mbias = np.full(SQ, NEG, f)
        mbias[:ns] = 0.0
        cfm = np.concatenate([
            np.asarray(bq)[gs].astype(f).reshape(JT, 128).T,
            np.asarray(bk)[gs].astype(f).reshape(JT, 128).T,
            mbias.reshape(st_tiles, 128).T,
        ], axis=1)
        in_maps.append({
            "xqT": np.ascontiguousarray(np.asarray(query[b]).T, dtype=bf),
            "xkT": xk,
            "xvT": xv,
            "wqT": np.ascontiguousarray(np.asarray(Wq)[gs, :].T, dtype=bf),
            "wkT": np.ascontiguousarray(np.asarray(Wk)[gs, :].T, dtype=bf),
            "wvT": np.ascontiguousarray(np.asarray(Wv)[gs, :].T, dtype=bf),
            "woT": np.ascontiguousarray(np.asarray(Wo)[:, gs].T, dtype=bf),
            "cf": np.ascontiguousarray(cfm),
            "bvr": np.ascontiguousarray(np.asarray(bv)[gs].astype(bf).reshape(1, G)),
            "idn": ident,
        })
    return in_maps


def _needed_st_tiles(key_padding_mask, local_mask):
    worst = 0
    for c in range(8):
        b, g = c // 2, c % 2
        mask = np.asarray((key_padding_mask if g == 0 else local_mask)[b])
        worst = max(worst, int((~mask).sum()))
    return max(1, -(-worst // 128))


def kernel(query, key, value, key_padding_mask, local_mask,
           Wq, bq, Wk, bk, Wv, bv, Wo, bo, _trace=False, _tmpdir=None):
    from concourse.bass_utils import run_bass_kernel_spmd

    st = min(max(_needed_st_tiles(key_padding_mask, local_mask), 5), 8)
    nc = _get_nc(st)
    in_maps = make_in_maps(query, key, value, key_padding_mask, local_mask,
                           Wq, bq, Wk, bk, Wv, bv, Wo, bo, st_tiles=st)
    try:
        res = run_bass_kernel_spmd(nc, in_maps, list(range(8)),
                                   trace=_trace, tmpdir=_tmpdir)
    except Exception:
        # transient device/transport failures have been observed on the
        # axon path; one fresh attempt is cheap relative to a hard fail
        res = run_bass_kernel_spmd(nc, in_maps, list(range(8)),
                                   trace=_trace, tmpdir=_tmpdir)
    outs = [np.asarray(r["out"]) for r in res.results]
    full = np.stack([outs[2 * b] + outs[2 * b + 1] for b in range(B)])
    full += np.asarray(bo, dtype=np.float32)
    if _trace:
        kernel._last_exec_time_ns = res.exec_time_ns
        kernel._last_profile = res.profile_json
    return full.astype(np.float32)


# revision 13
# speedup vs baseline: 1.3087x; 1.0003x over previous
"""Cross multihead attention (global/local masked head groups) on 8 trn2 cores.

Sharding: core c -> (batch b = c//2, head-group g = c%2).
  g=0: heads 0-7  masked by key_padding_mask[b]
  g=1: heads 8-15 masked by local_mask[b]
Each core computes its group's partial output (attn_out_g @ Wo[:, gs].T)
of shape [T, E]; the host sums the two partials per batch and adds bo.

Key structure:
  - Masked keys contribute nothing (exp -> 0), so the host gathers the
    ~50% unmasked key/value columns into an SQ=640-padded buffer;
    k/v projections, QK^T and AV all shrink accordingly.  Padding
    columns keep a -30000 exp bias so they vanish from the softmax.
  - scoresT orientation [s, t]: mask bias is per-partition for the exp.
  - AV is computed as out[t, d] = expT.T @ [v | ones]: the moving operand
    is only 65 wide (PE cost ~ moving free size), and the ones column
    gives the softmax denominator per t-partition, so normalization is
    a per-partition scalar multiply (no partition broadcast needed).
  - A PE transpose (identity moving operand) flips normalized [t, j]
    head pairs back to [j, t] for the output projection.
  - All big DRAM loads are single batched DMAs (HWDGE serializes at
    ~625ns per dma_start).
  - Software-pipelined emission: the PE stream interleaves "A units"
    (q-proj + QK + exp, which pace at the Act engine's exp speed via
    PSUM buffer rotation) with "B units" (v-proj, AV chains, output
    projection) so the PE never idles while Act catches up.
"""

import os
import sys

sys.path.insert(0, "/opt/trn_rl_repo")

import numpy as np

import concourse.bass as bass
import concourse.mybir as mybir
from concourse.tile import TileContext

B, T, S, E, H = 4, 1024, 1024, 1024, 16
DH = E // H            # 64
HH = H // 2            # 8 heads per group
G = HH * DH            # 512 features per group
SCALING = DH ** -0.5
NEG = -30000.0         # exp(x + NEG) == 0.0 in fp32, no LUT edge cases

F32 = mybir.dt.float32
BF = mybir.dt.bfloat16

ET = E // 128          # 8 contraction tiles
JT = G // 128          # 4 j-tiles (head pairs)
TT = T // 128          # 8 t tiles


def _split_waits(nc):
    """TPB ISA structs hold one sem-wait slot. Tile can emit >1 wait per
    instruction (walrus: 'Too many sync wait commands'); hoist all but the
    last wait onto single-wait NOPs on the same engine, inserted just
    before. Timing is unchanged - the waits would have blocked anyway."""
    k = 0
    for f in nc.m.functions:
        for blk in f.blocks:
            new = []
            for inst in blk.instructions:
                si = inst.sync_info
                w = list(si.on_wait) if si else []
                if len(w) > 1:
                    for wait in w[:-1]:
                        nop = mybir.InstNoOp(name=f"nopw-{k}", ins=[], outs=[])
                        k += 1
                        nop.engine = inst.engine
                        nop.sync_info = mybir.SyncInfo(on_wait=[wait], on_update=[])
                        new.append(nop)
                    inst.sync_info = mybir.SyncInfo(
                        on_wait=[w[-1]], on_update=list(si.on_update)
                    )
                new.append(inst)
            blk.instructions = new
    return nc


def build_nc(st_tiles=5, split=True, off=11):
    ST = st_tiles          # number of 128-wide s tiles after compression
    SQ = ST * 128          # padded compressed key count

    nc = bass.Bass()

    xqT = nc.dram_tensor("xqT", [E, T], BF, kind="ExternalInput")
    xkT = nc.dram_tensor("xkT", [E, SQ], BF, kind="ExternalInput")
    xvT = nc.dram_tensor("xvT", [E, SQ], BF, kind="ExternalInput")
    wqT = nc.dram_tensor("wqT", [E, G], BF, kind="ExternalInput")
    wkT = nc.dram_tensor("wkT", [E, G], BF, kind="ExternalInput")
    wvT = nc.dram_tensor("wvT", [E, G], BF, kind="ExternalInput")
    woT = nc.dram_tensor("woT", [G, E], BF, kind="ExternalInput")
    # packed f32 consts: bq | bk | mb  -> [128, 4 + 4 + ST]
    cf = nc.dram_tensor("cf", [128, 2 * JT + ST], F32, kind="ExternalInput")
    bvr = nc.dram_tensor("bvr", [1, G], BF, kind="ExternalInput")
    idn = nc.dram_tensor("idn", [128, 128], BF, kind="ExternalInput")
    out = nc.dram_tensor("out", [T, E], F32, kind="ExternalOutput")

    with TileContext(nc) as tc:
        with (
            tc.tile_pool(name="const", bufs=1) as pc,
            tc.tile_pool(name="persist", bufs=1) as pp,
            tc.tile_pool(name="exp", bufs=HH * ST) as pe,
            tc.tile_pool(name="small", bufs=4) as psm,
            tc.tile_pool(name="outsb", bufs=2) as po,
            tc.tile_pool(name="ps", bufs=2, space="PSUM") as pps,
        ):
            # ---- staged loads, startup-latency ordered ----
            # wk/xk land in quarters so the k projection starts ~3us in
            wk_sb = pc.tile([128, ET * G], BF, name="wk_sb")
            xk_sb = pc.tile([128, ET * SQ], BF, name="xk_sb")
            EH = ET // 4
            for eh in range(4):
                nc.sync.dma_start(
                    out=wk_sb[:].rearrange("p (e t) -> p e t", e=ET)[:, eh * EH:(eh + 1) * EH],
                    in_=wkT[:].rearrange("(e p) t -> p e t", p=128)[:, eh * EH:(eh + 1) * EH],
                )
                nc.sync.dma_start(
                    out=xk_sb[:].rearrange("p (e t) -> p e t", e=ET)[:, eh * EH:(eh + 1) * EH],
                    in_=xkT[:].rearrange("(e p) t -> p e t", p=128)[:, eh * EH:(eh + 1) * EH],
                )
            cf_sb = pc.tile([128, 2 * JT + ST], F32, name="cf_sb")
            nc.sync.dma_start(out=cf_sb[:], in_=cf[:])
            bq_sb = cf_sb[:, 0:JT]
            bk_sb = cf_sb[:, JT:2 * JT]
            mb_sb = cf_sb[:, 2 * JT:]

            wq_sb = pc.tile([128, ET * G], BF, name="wq_sb")
            nc.sync.dma_start(
                out=wq_sb[:].rearrange("p (e t) -> p e t", e=ET),
                in_=wqT[:].rearrange("(e p) t -> p e t", p=128),
            )
            xq_sb = pc.tile([128, ET * T], BF, name="xq_sb")
            nc.sync.dma_start(
                out=xq_sb[:].rearrange("p (e t) -> p e t", e=ET),
                in_=xqT[:].rearrange("(e p) t -> p e t", p=128),
            )
            bv_sb = pc.tile([1, G], BF, name="bv_sb")
            nc.sync.dma_start(out=bv_sb[:], in_=bvr[:])
            id_sb = pc.tile([128, 128], BF, name="id_sb")
            nc.sync.dma_start(out=id_sb[:], in_=idn[:])
            wv_sb = pc.tile([128, ET * G], BF, name="wv_sb")
            nc.sync.dma_start(
                out=wv_sb[:].rearrange("p (e t) -> p e t", e=ET),
                in_=wvT[:].rearrange("(e p) t -> p e t", p=128),
            )
            xv_sb = pc.tile([128, ET * SQ], BF, name="xv_sb")
            nc.sync.dma_start(
                out=xv_sb[:].rearrange("p (e t) -> p e t", e=ET),
                in_=xvT[:].rearrange("(e p) t -> p e t", p=128),
            )
            wo_sb = pc.tile([128, JT * E], BF, name="wo_sb")
            nc.sync.dma_start(
                out=wo_sb[:].rearrange("p (r t) -> p r t", r=JT),
                in_=woT[:].rearrange("(r p) t -> p r t", p=128),
            )
            ones_sb = pc.tile([1, 128], BF, name="ones_sb")
            nc.gpsimd.memset(ones_sb[:], 1.0)

            # PE p-state warmup: burn the 3us clock ramp on dummy matmuls
            # while the first input DMAs land, so real work starts at 2.4GHz.
            warm_sb = pc.tile([128, 512], BF, name="warm_sb")
            nc.gpsimd.memset(warm_sb[:], 0.0)
            for w in range(9):
                ps_w = pps.tile([128, 512], F32, tag="aux", name="ps_w")
                nc.tensor.matmul(
                    ps_w[:], lhsT=warm_sb[:, 0:128], rhs=warm_sb[:],
                    start=True, stop=True,
                )

            xk3 = xk_sb[:].rearrange("p (e t) -> p e t", e=ET)
            wk3 = wk_sb[:].rearrange("p (e t) -> p e t", e=ET)
            xq3 = xq_sb[:].rearrange("p (e t) -> p e t", e=ET)
            wq3 = wq_sb[:].rearrange("p (e t) -> p e t", e=ET)
            xv3 = xv_sb[:].rearrange("p (e t) -> p e t", e=ET)
            wv3 = wv_sb[:].rearrange("p (e t) -> p e t", e=ET)
            wo3 = wo_sb[:].rearrange("p (r t) -> p r t", r=JT)

            # ---- persistent activations ----
            qT_sb = [pp.tile([128, T], BF, name=f"qT{r}") for r in range(JT)]
            kT_sb = [pp.tile([128, SQ], BF, name=f"kT{r}") for r in range(JT)]
            v_sb = [pp.tile([128, HH * (DH + 1)], BF, name=f"v{st}") for st in range(ST)]
            aT_sb = [pp.tile([128, T], BF, name=f"aT{r}") for r in range(JT)]
            expT = [None] * (HH * ST)

            # ---- k projection (pure PE warmup while other DMAs land) ----
            for r in range(JT):
                for lo, hi in ((0, 512), (512, SQ)):
                    if lo >= SQ:
                        continue
                    ps = pps.tile([128, 512], F32, tag="aux", name="ps_k")
                    for et in range(ET):
                        nc.tensor.matmul(
                            ps[:, 0:hi - lo],
                            lhsT=wk3[:, et, r * 128:(r + 1) * 128],
                            rhs=xk3[:, et, lo:hi],
                            start=(et == 0), stop=(et == ET - 1),
                        )
                    nc.vector.tensor_scalar_add(
                        kT_sb[r][:, lo:hi], ps[:, 0:hi - lo], bk_sb[:, r:r + 1]
                    )

            # ---- unit emitters ----
            def emit_q(r):
                for lo, hi in ((0, 512), (512, 1024)):
                    ps = pps.tile([128, 512], F32, tag="aux", name="ps_q")
                    for et in range(ET):
                        nc.tensor.matmul(
                            ps[:],
                            lhsT=wq3[:, et, r * 128:(r + 1) * 128],
                            rhs=xq3[:, et, lo:hi],
                            start=(et == 0), stop=(et == ET - 1),
                        )
                    nc.vector.tensor_scalar_add(
                        qT_sb[r][:, lo:hi], ps[:], bq_sb[:, r:r + 1]
                    )

            def emit_qk(r, hl, st):
                h = 2 * r + hl
                po_ = hl * DH
                ps_s = pps.tile([128, 1024], F32, tag="qk", name="ps_s")
                for lo, hi in ((0, 512), (512, 1024)):
                    nc.tensor.matmul(
                        ps_s[:, lo:hi],
                        lhsT=kT_sb[r][po_:po_ + DH, st * 128:(st + 1) * 128],
                        rhs=qT_sb[r][po_:po_ + DH, lo:hi],
                        start=True, stop=True,
                    )
                ex = pe.tile([128, T], BF, tag="exp", name=f"exp{h}_{st}")
                expT[h * ST + st] = ex
                nc.scalar.activation(
                    ex[:], ps_s[:],
                    mybir.ActivationFunctionType.Exp,
                    bias=mb_sb[:, st:st + 1], scale=SCALING,
                )

            bvb_sb = pc.tile([128, G], BF, name="bvb_sb")

            def emit_v(st):
                if st == 0:
                    psb = pps.tile([128, 512], F32, tag="aux", name="ps_bvb")
                    nc.tensor.matmul(  # bv broadcast: ones[1,128].T @ bv[1,512]
                        psb[:, 0:G], lhsT=ones_sb[:], rhs=bv_sb[:],
                        start=True, stop=True,
                    )
                    nc.vector.tensor_copy(bvb_sb[:], psb[:, 0:G])
                ps = pps.tile([128, 512], F32, tag="aux", name="ps_v")
                for et in range(ET):
                    nc.tensor.matmul(
                        ps[:, 0:G],
                        lhsT=xv3[:, et, st * 128:(st + 1) * 128],
                        rhs=wv3[:, et, :],
                        start=(et == 0), stop=(et == ET - 1),
                    )
                v3 = v_sb[st][:].rearrange("p (h x) -> p h x", x=DH + 1)
                nc.vector.tensor_tensor(
                    v3[:, :, 0:DH],
                    ps[:, 0:G].rearrange("p (h x) -> p h x", x=DH),
                    bvb_sb[:].rearrange("p (h x) -> p h x", x=DH),
                    op=mybir.AluOpType.add,
                )
                nc.gpsimd.memset(v3[:, :, DH:DH + 1], 1.0)

            pav_cur = [None, None]
            op_cur = [None]

            def emit_av(hp, tt, i, defer_tr=False):
                h = 2 * hp + i
                pav = pps.tile([128, 512], F32, tag="avh", name="pav")
                pav_cur[i] = pav
                for st in range(ST):
                    nc.tensor.matmul(
                        pav[:, 0:DH + 1],
                        lhsT=expT[h * ST + st][:, tt * 128:(tt + 1) * 128],
                        rhs=v_sb[st][:, h * (DH + 1):(h + 1) * (DH + 1)],
                        start=(st == 0), stop=(st == ST - 1),
                    )
                if i == 0:
                    op_cur[0] = psm.tile([128, 128], BF, tag="op", name="op")
                op = op_cur[0]
                nc.vector.tensor_scalar(
                    op[:, i * DH:(i + 1) * DH], pav[:, 0:DH],
                    pav[:, DH:DH + 1], None, op0=mybir.AluOpType.divide,
                )
                if i == 1 and not defer_tr:
                    emit_trc(hp, tt)

            def emit_trc(hp, tt):
                op = op_cur[0]
                tr = pps.tile([128, 128], BF, tag="aux", name="tr")
                nc.tensor.transpose(tr[:], op[:], id_sb[:])
                nc.vector.tensor_copy(
                    aT_sb[hp][:, tt * 128:(tt + 1) * 128], tr[:]
                )

            def emit_op(t2):
                osb = po.tile([128, E], F32, tag="osb", name="osb")
                for oc in range(2):
                    pu = pps.tile([128, 512], F32, tag="aux", name="pu")
                    for r in range(JT):
                        nc.tensor.matmul(
                            pu[:, 0:512],
                            lhsT=aT_sb[r][:, t2 * 128:(t2 + 1) * 128],
                            rhs=wo3[:, r, oc * 512:(oc + 1) * 512],
                            start=(r == 0), stop=(r == JT - 1),
                        )
                    if oc == 0:
                        nc.vector.tensor_copy(osb[:, 0:512], pu[:, 0:512])
                    else:
                        nc.scalar.copy(osb[:, 512:1024], pu[:, 0:512])
                    nc.sync.dma_start(
                        out=out[t2 * 128:(t2 + 1) * 128,
                                oc * 512:(oc + 1) * 512],
                        in_=osb[:, oc * 512:(oc + 1) * 512],
                    )

            # ---- interleaved A/B emission ----
            a_list = []
            for r in range(JT):
                a_list.append(("q", r))
                for hl in range(2):
                    for st in range(ST):
                        a_list.append(("qk", r, hl, st))
            b_list = [("v", st) for st in range(ST)]
            for hp in range(JT):
                for tt in range(TT):
                    b_list.append(("av", hp, tt, 0))
                    if hp == JT - 1:
                        b_list.append(("avd", hp, tt, 1))
                        if tt >= 1:
                            b_list.append(("op", tt - 1))
                        b_list.append(("trc", hp, tt))
                    else:
                        b_list.append(("av", hp, tt, 1))
            b_list.append(("op", TT - 1))

            def emit(u):
                kind = u[0]
                if kind == "q":
                    emit_q(u[1])
                elif kind == "qk":
                    emit_qk(u[1], u[2], u[3])
                elif kind == "v":
                    emit_v(u[1])
                elif kind == "av":
                    emit_av(u[1], u[2], u[3])
                elif kind == "avd":
                    emit_av(u[1], u[2], u[3], defer_tr=True)
                elif kind == "trc":
                    emit_trc(u[1], u[2])
                elif kind == "op":
                    emit_op(u[1])

            def a_idx(r, hl, st):
                return r * (2 * ST + 1) + 1 + hl * ST + st

            ua = 2 * ST + 1   # A-units per j-tile

            def min_a(u):
                if u[0] == "v":
                    return off + 1
                if u[0] in ("av", "avd"):
                    _, hp, tt, i = u
                    spread = off + 1 + ua * hp + ((tt * 2 + i) * ua) // 16
                    return max(a_idx(hp, i, ST - 1), spread)
                if u[0] == "trc":
                    return a_idx(u[1], 1, ST - 1)
                if u[0] == "op":
                    return a_idx(JT - 1, 1, ST - 1)
                return 0

            nA, nB = len(a_list), len(b_list)
            ib = 0
            for i, u in enumerate(a_list):
                emit(u)
                while ib < nB and min_a(b_list[ib]) <= i:
                    emit(b_list[ib])
                    ib += 1
            while ib < nB:
                emit(b_list[ib])
                ib += 1
    return _split_waits(nc) if split else nc


_NC_CACHE = {}


def _get_nc(st_tiles=5):
    if st_tiles not in _NC_CACHE:
        _NC_CACHE[st_tiles] = build_nc(st_tiles)
    return _NC_CACHE[st_tiles]


def make_in_maps(query, key, value, key_padding_mask, local_mask,
                 Wq, bq, Wk, bk, Wv, bv, Wo, bo, st_tiles=5):
    import ml_dtypes
    f = np.float32
    bf = ml_dtypes.bfloat16
    SQ = st_tiles * 128
    ident = np.eye(128, dtype=bf)
    in_maps = []
    for c in range(8):
        b, g = c // 2, c % 2
        gs = slice(g * G, (g + 1) * G)
        mask = np.asarray((key_padding_mask if g == 0 else local_mask)[b])
        sel = np.flatnonzero(~mask)
        ns = sel.size
        assert ns <= SQ, (ns, SQ)
        xk = np.zeros((E, SQ), dtype=bf)
        xk[:, :ns] = np.asarray(key[b]).T[:, sel]
        xv = np.zeros((E, SQ), dtype=bf)
        xv[:, :ns] = np.asarray(value[b]).T[:, sel]
        mbias = np.full(SQ, NEG, f)
        mbias[:ns] = 0.0
        cfm = np.concatenate([
            np.asarray(bq)[gs].astype(f).reshape(JT, 128).T,
            np.asarray(bk)[gs].astype(f).reshape(JT, 128).T,
            mbias.reshape(st_tiles, 128).T,
        ], axis=1)
        in_maps.append({
            "xqT": np.ascontiguousarray(np.asarray(query[b]).T, dtype=bf),
            "xkT": xk,
            "xvT": xv,
            "wqT": np.ascontiguousarray(np.asarray(Wq)[gs, :].T, dtype=bf),
            "wkT": np.ascontiguousarray(np.asarray(Wk)[gs, :].T, dtype=bf),
            "wvT": np.ascontiguousarray(np.asarray(Wv)[gs, :].T, dtype=bf),
            "woT": np.ascontiguousarray(np.asarray(Wo)[:, gs].T, dtype=bf),
            "cf": np.ascontiguousarray(cfm),
            "bvr": np.ascontiguousarray(np.asarray(bv)[gs].astype(bf).reshape(1, G)),
            "idn": ident,
        })
    return in_maps


def _needed_st_tiles(key_padding_mask, local_mask):
    worst = 0
    for c in range(8):
        b, g = c // 2, c % 2
        mask = np.asarray((key_padding_mask if g == 0 else local_mask)[b])
        worst = max(worst, int((~mask).sum()))
    return max(1, -(-worst // 128))


def kernel(query, key, value, key_padding_mask, local_mask,
           Wq, bq, Wk, bk, Wv, bv, Wo, bo, _trace=False, _tmpdir=None):
    from concourse.bass_utils import run_bass_kernel_spmd

    st = min(max(_needed_st_tiles(key_padding_mask, local_mask), 5), 8)
    nc = _get_nc(st)
    in_maps = make_in_maps(query, key, value, key_padding_mask, local_mask,
                           Wq, bq, Wk, bk, Wv, bv, Wo, bo, st_tiles=st)
    try:
        res = run_bass_kernel_spmd(nc, in_maps, list(range(8)),
                                   trace=_trace, tmpdir=_tmpdir)
    except Exception:
        # transient device/transport failures have been observed on the
        # axon path; one fresh attempt is cheap relative to a hard fail
        res = run_bass_kernel_spmd(nc, in_maps, list(range(8)),
                                   trace=_trace, tmpdir=_tmpdir)
    outs = [np.asarray(r["out"]) for r in res.results]
    full = np.stack([outs[2 * b] + outs[2 * b + 1] for b in range(B)])
    full += np.asarray(bo, dtype=np.float32)
    if _trace:
        kernel._last_exec_time_ns = res.exec_time_ns
        kernel._last_profile = res.profile_json
    return full.astype(np.float32)


# revision 15
# speedup vs baseline: 1.3256x; 1.0129x over previous
"""Cross multihead attention (global/local masked head groups) on 8 trn2 cores.

Sharding: core c -> (batch b = c//2, head-group g = c%2).
  g=0: heads 0-7  masked by key_padding_mask[b]
  g=1: heads 8-15 masked by local_mask[b]
Each core computes its group's partial output (attn_out_g @ Wo[:, gs].T)
of shape [T, E]; the host sums the two partials per batch and adds bo.

Key structure:
  - Masked keys contribute nothing (exp -> 0), so the host gathers the
    ~50% unmasked key/value columns into an SQ=640-padded buffer;
    k/v projections, QK^T and AV all shrink accordingly.  Padding
    columns keep a -30000 exp bias so they vanish from the softmax.
  - scoresT orientation [s, t]: mask bias is per-partition for the exp.
  - AV is computed as out[t, d] = expT.T @ [v | ones]: the moving operand
    is only 65 wide (PE cost ~ moving free size), and the ones column
    gives the softmax denominator per t-partition, so normalization is
    a per-partition scalar multiply (no partition broadcast needed).
  - A PE transpose (identity moving operand) flips normalized [t, j]
    head pairs back to [j, t] for the output projection.
  - All big DRAM loads are single batched DMAs (HWDGE serializes at
    ~625ns per dma_start).
  - Software-pipelined emission: the PE stream interleaves "A units"
    (q-proj + QK + exp, which pace at the Act engine's exp speed via
    PSUM buffer rotation) with "B units" (v-proj, AV chains, output
    projection) so the PE never idles while Act catches up.
"""

import os
import sys

sys.path.insert(0, "/opt/trn_rl_repo")

import numpy as np

import concourse.bass as bass
import concourse.mybir as mybir
from concourse.tile import TileContext

B, T, S, E, H = 4, 1024, 1024, 1024, 16
DH = E // H            # 64
HH = H // 2            # 8 heads per group
G = HH * DH            # 512 features per group
SCALING = DH ** -0.5
NEG = -30000.0         # exp(x + NEG) == 0.0 in fp32, no LUT edge cases

F32 = mybir.dt.float32
BF = mybir.dt.bfloat16

ET = E // 128          # 8 contraction tiles
JT = G // 128          # 4 j-tiles (head pairs)
TT = T // 128          # 8 t tiles


def _split_waits(nc):
    """TPB ISA structs hold one sem-wait slot. Tile can emit >1 wait per
    instruction (walrus: 'Too many sync wait commands'); hoist all but the
    last wait onto single-wait NOPs on the same engine, inserted just
    before. Timing is unchanged - the waits would have blocked anyway."""
    k = 0
    for f in nc.m.functions:
        for blk in f.blocks:
            new = []
            for inst in blk.instructions:
                si = inst.sync_info
                w = list(si.on_wait) if si else []
                if len(w) > 1:
                    for wait in w[:-1]:
                        nop = mybir.InstNoOp(name=f"nopw-{k}", ins=[], outs=[])
                        k += 1
                        nop.engine = inst.engine
                        nop.sync_info = mybir.SyncInfo(on_wait=[wait], on_update=[])
                        new.append(nop)
                    inst.sync_info = mybir.SyncInfo(
                        on_wait=[w[-1]], on_update=list(si.on_update)
                    )
                new.append(inst)
            blk.instructions = new
    return nc


def build_nc(st_tiles=5, split=True, off=19):
    ST = st_tiles          # number of 128-wide s tiles after compression
    SQ = ST * 128          # padded compressed key count

    nc = bass.Bass()

    xqT = nc.dram_tensor("xqT", [E, T], BF, kind="ExternalInput")
    xkT = nc.dram_tensor("xkT", [E, SQ], BF, kind="ExternalInput")
    xvT = nc.dram_tensor("xvT", [E, SQ], BF, kind="ExternalInput")
    wqT = nc.dram_tensor("wqT", [E, G], BF, kind="ExternalInput")
    wkT = nc.dram_tensor("wkT", [E, G], BF, kind="ExternalInput")
    wvT = nc.dram_tensor("wvT", [E, G], BF, kind="ExternalInput")
    woT = nc.dram_tensor("woT", [G, E], BF, kind="ExternalInput")
    # packed f32 consts: bq | bk | mb  -> [128, 4 + 4 + ST]
    cf = nc.dram_tensor("cf", [128, 2 * JT + ST], F32, kind="ExternalInput")
    bvr = nc.dram_tensor("bvr", [1, G], BF, kind="ExternalInput")
    idn = nc.dram_tensor("idn", [128, 128], BF, kind="ExternalInput")
    out = nc.dram_tensor("out", [T, E], F32, kind="ExternalOutput")

    with TileContext(nc) as tc:
        with (
            tc.tile_pool(name="const", bufs=1) as pc,
            tc.tile_pool(name="persist", bufs=1) as pp,
            tc.tile_pool(name="exp", bufs=HH * ST) as pe,
            tc.tile_pool(name="small", bufs=4) as psm,
            tc.tile_pool(name="outsb", bufs=2) as po,
            tc.tile_pool(name="ps", bufs=2, space="PSUM") as pps,
        ):
            # ---- staged loads, startup-latency ordered ----
            # wk/xk land in quarters so the k projection starts ~3us in
            wk_sb = pc.tile([128, ET * G], BF, name="wk_sb")
            xk_sb = pc.tile([128, ET * SQ], BF, name="xk_sb")
            EH = ET // 4
            for eh in range(4):
                nc.sync.dma_start(
                    out=wk_sb[:].rearrange("p (e t) -> p e t", e=ET)[:, eh * EH:(eh + 1) * EH],
                    in_=wkT[:].rearrange("(e p) t -> p e t", p=128)[:, eh * EH:(eh + 1) * EH],
                )
                nc.sync.dma_start(
                    out=xk_sb[:].rearrange("p (e t) -> p e t", e=ET)[:, eh * EH:(eh + 1) * EH],
                    in_=xkT[:].rearrange("(e p) t -> p e t", p=128)[:, eh * EH:(eh + 1) * EH],
                )
            cf_sb = pc.tile([128, 2 * JT + ST], F32, name="cf_sb")
            nc.sync.dma_start(out=cf_sb[:], in_=cf[:])
            bq_sb = cf_sb[:, 0:JT]
            bk_sb = cf_sb[:, JT:2 * JT]
            mb_sb = cf_sb[:, 2 * JT:]

            wq_sb = pc.tile([128, ET * G], BF, name="wq_sb")
            nc.sync.dma_start(
                out=wq_sb[:].rearrange("p (e t) -> p e t", e=ET),
                in_=wqT[:].rearrange("(e p) t -> p e t", p=128),
            )
            xq_sb = pc.tile([128, ET * T], BF, name="xq_sb")
            nc.sync.dma_start(
                out=xq_sb[:].rearrange("p (e t) -> p e t", e=ET),
                in_=xqT[:].rearrange("(e p) t -> p e t", p=128),
            )
            bv_sb = pc.tile([1, G], BF, name="bv_sb")
            nc.sync.dma_start(out=bv_sb[:], in_=bvr[:])
            id_sb = pc.tile([128, 128], BF, name="id_sb")
            nc.sync.dma_start(out=id_sb[:], in_=idn[:])
            wv_sb = pc.tile([128, ET * G], BF, name="wv_sb")
            nc.sync.dma_start(
                out=wv_sb[:].rearrange("p (e t) -> p e t", e=ET),
                in_=wvT[:].rearrange("(e p) t -> p e t", p=128),
            )
            xv_sb = pc.tile([128, ET * SQ], BF, name="xv_sb")
            nc.sync.dma_start(
                out=xv_sb[:].rearrange("p (e t) -> p e t", e=ET),
                in_=xvT[:].rearrange("(e p) t -> p e t", p=128),
            )
            wo_sb = pc.tile([128, JT * E], BF, name="wo_sb")
            nc.sync.dma_start(
                out=wo_sb[:].rearrange("p (r t) -> p r t", r=JT),
                in_=woT[:].rearrange("(r p) t -> p r t", p=128),
            )
            ones_sb = pc.tile([1, 128], BF, name="ones_sb")
            nc.gpsimd.memset(ones_sb[:], 1.0)

            # PE p-state warmup: burn the 3us clock ramp on dummy matmuls
            # while the first input DMAs land, so real work starts at 2.4GHz.
            warm_sb = pc.tile([128, 512], BF, name="warm_sb")
            nc.gpsimd.memset(warm_sb[:], 0.0)
            for w in range(9):
                ps_w = pps.tile([128, 512], F32, tag="aux", name="ps_w")
                nc.tensor.matmul(
                    ps_w[:], lhsT=warm_sb[:, 0:128], rhs=warm_sb[:],
                    start=True, stop=True,
                )

            xk3 = xk_sb[:].rearrange("p (e t) -> p e t", e=ET)
            wk3 = wk_sb[:].rearrange("p (e t) -> p e t", e=ET)
            xq3 = xq_sb[:].rearrange("p (e t) -> p e t", e=ET)
            wq3 = wq_sb[:].rearrange("p (e t) -> p e t", e=ET)
            xv3 = xv_sb[:].rearrange("p (e t) -> p e t", e=ET)
            wv3 = wv_sb[:].rearrange("p (e t) -> p e t", e=ET)
            wo3 = wo_sb[:].rearrange("p (r t) -> p r t", r=JT)

            # ---- persistent activations ----
            qT_sb = [pp.tile([128, T], BF, name=f"qT{r}") for r in range(JT)]
            kT_sb = [pp.tile([128, SQ], BF, name=f"kT{r}") for r in range(JT)]
            v_sb = [pp.tile([128, HH * (DH + 1)], BF, name=f"v{st}") for st in range(ST)]
            aT_sb = [pp.tile([128, T], BF, name=f"aT{r}") for r in range(JT)]
            expT = [None] * (HH * ST)

            # ---- k projection (pure PE warmup while other DMAs land) ----
            for r in range(JT):
                for lo, hi in ((0, 512), (512, SQ)):
                    if lo >= SQ:
                        continue
                    ps = pps.tile([128, 512], F32, tag="aux", name="ps_k")
                    for et in range(ET):
                        nc.tensor.matmul(
                            ps[:, 0:hi - lo],
                            lhsT=wk3[:, et, r * 128:(r + 1) * 128],
                            rhs=xk3[:, et, lo:hi],
                            start=(et == 0), stop=(et == ET - 1),
                        )
                    nc.vector.tensor_scalar_add(
                        kT_sb[r][:, lo:hi], ps[:, 0:hi - lo], bk_sb[:, r:r + 1]
                    )

            # ---- unit emitters ----
            def emit_q(r):
                for lo, hi in ((0, 512), (512, 1024)):
                    ps = pps.tile([128, 512], F32, tag="aux", name="ps_q")
                    for et in range(ET):
                        nc.tensor.matmul(
                            ps[:],
                            lhsT=wq3[:, et, r * 128:(r + 1) * 128],
                            rhs=xq3[:, et, lo:hi],
                            start=(et == 0), stop=(et == ET - 1),
                        )
                    nc.vector.tensor_scalar_add(
                        qT_sb[r][:, lo:hi], ps[:], bq_sb[:, r:r + 1]
                    )

            def emit_qk(r, hl, st):
                h = 2 * r + hl
                po_ = hl * DH
                ps_s = pps.tile([128, 1024], F32, tag="qk", name="ps_s")
                for lo, hi in ((0, 512), (512, 1024)):
                    nc.tensor.matmul(
                        ps_s[:, lo:hi],
                        lhsT=kT_sb[r][po_:po_ + DH, st * 128:(st + 1) * 128],
                        rhs=qT_sb[r][po_:po_ + DH, lo:hi],
                        start=True, stop=True,
                    )
                ex = pe.tile([128, T], BF, tag="exp", name=f"exp{h}_{st}")
                expT[h * ST + st] = ex
                nc.scalar.activation(
                    ex[:], ps_s[:],
                    mybir.ActivationFunctionType.Exp,
                    bias=mb_sb[:, st:st + 1], scale=SCALING,
                )

            bvb_sb = pc.tile([128, G], BF, name="bvb_sb")

            def emit_v(st):
                if st == 0:
                    psb = pps.tile([128, 512], F32, tag="aux", name="ps_bvb")
                    nc.tensor.matmul(  # bv broadcast: ones[1,128].T @ bv[1,512]
                        psb[:, 0:G], lhsT=ones_sb[:], rhs=bv_sb[:],
                        start=True, stop=True,
                    )
                    nc.vector.tensor_copy(bvb_sb[:], psb[:, 0:G])
                ps = pps.tile([128, 512], F32, tag="aux", name="ps_v")
                for et in range(ET):
                    nc.tensor.matmul(
                        ps[:, 0:G],
                        lhsT=xv3[:, et, st * 128:(st + 1) * 128],
                        rhs=wv3[:, et, :],
                        start=(et == 0), stop=(et == ET - 1),
                    )
                v3 = v_sb[st][:].rearrange("p (h x) -> p h x", x=DH + 1)
                nc.vector.tensor_tensor(
                    v3[:, :, 0:DH],
                    ps[:, 0:G].rearrange("p (h x) -> p h x", x=DH),
                    bvb_sb[:].rearrange("p (h x) -> p h x", x=DH),
                    op=mybir.AluOpType.add,
                )
                nc.gpsimd.memset(v3[:, :, DH:DH + 1], 1.0)

            pav_cur = [None, None]
            op_cur = [None]

            def emit_av(hp, tt, i, defer_tr=False):
                h = 2 * hp + i
                pav = pps.tile([128, 512], F32, tag="avh", name="pav")
                pav_cur[i] = pav
                for st in range(ST):
                    nc.tensor.matmul(
                        pav[:, 0:DH + 1],
                        lhsT=expT[h * ST + st][:, tt * 128:(tt + 1) * 128],
                        rhs=v_sb[st][:, h * (DH + 1):(h + 1) * (DH + 1)],
                        start=(st == 0), stop=(st == ST - 1),
                    )
                if i == 0:
                    op_cur[0] = psm.tile([128, 128], BF, tag="op", name="op")
                op = op_cur[0]
                rec = psm.tile([128, 1], F32, tag="rec", name="rec")
                nc.vector.reciprocal(rec[:], pav[:, DH:DH + 1])
                nc.vector.tensor_scalar_mul(
                    op[:, i * DH:(i + 1) * DH], pav[:, 0:DH], rec[:, 0:1]
                )
                if i == 1 and not defer_tr:
                    emit_trc(hp, tt)

            def emit_trc(hp, tt):
                op = op_cur[0]
                tr = pps.tile([128, 128], BF, tag="aux", name="tr")
                nc.tensor.transpose(tr[:], op[:], id_sb[:])
                nc.vector.tensor_copy(
                    aT_sb[hp][:, tt * 128:(tt + 1) * 128], tr[:]
                )

            def emit_op(t2):
                osb = po.tile([128, E], F32, tag="osb", name="osb")
                for oc in range(2):
                    pu = pps.tile([128, 512], F32, tag="aux", name="pu")
                    for r in range(JT):
                        nc.tensor.matmul(
                            pu[:, 0:512],
                            lhsT=aT_sb[r][:, t2 * 128:(t2 + 1) * 128],
                            rhs=wo3[:, r, oc * 512:(oc + 1) * 512],
                            start=(r == 0), stop=(r == JT - 1),
                        )
                    if oc == 0:
                        nc.vector.tensor_copy(osb[:, 0:512], pu[:, 0:512])
                    else:
                        nc.scalar.copy(osb[:, 512:1024], pu[:, 0:512])
                    nc.sync.dma_start(
                        out=out[t2 * 128:(t2 + 1) * 128,
                                oc * 512:(oc + 1) * 512],
                        in_=osb[:, oc * 512:(oc + 1) * 512],
                    )

            # ---- interleaved A/B emission ----
            a_list = []
            for r in range(JT):
                a_list.append(("q", r))
                for hl in range(2):
                    for st in range(ST):
                        a_list.append(("qk", r, hl, st))
            b_list = [("v", st) for st in range(ST)]
            for hp in range(JT):
                for tt in range(TT):
                    b_list.append(("av", hp, tt, 0))
                    if hp == JT - 1:
                        b_list.append(("avd", hp, tt, 1))
                        if tt >= 1:
                            b_list.append(("op", tt - 1))
                        b_list.append(("trc", hp, tt))
                    else:
                        b_list.append(("av", hp, tt, 1))
            b_list.append(("op", TT - 1))

            def emit(u):
                kind = u[0]
                if kind == "q":
                    emit_q(u[1])
                elif kind == "qk":
                    emit_qk(u[1], u[2], u[3])
                elif kind == "v":
                    emit_v(u[1])
                elif kind == "av":
                    emit_av(u[1], u[2], u[3])
                elif kind == "avd":
                    emit_av(u[1], u[2], u[3], defer_tr=True)
                elif kind == "trc":
                    emit_trc(u[1], u[2])
                elif kind == "op":
                    emit_op(u[1])

            def a_idx(r, hl, st):
                return r * (2 * ST + 1) + 1 + hl * ST + st

            ua = 2 * ST + 1   # A-units per j-tile

            def min_a(u):
                if u[0] == "v":
                    return off + 1
                if u[0] in ("av", "avd"):
                    _, hp, tt, i = u
                    spread = off + 1 + ua * hp + ((tt * 2 + i) * ua) // 16
                    return max(a_idx(hp, i, ST - 1), spread)
                if u[0] == "trc":
                    return a_idx(u[1], 1, ST - 1)
                if u[0] == "op":
                    return a_idx(JT - 1, 1, ST - 1)
                return 0

            nA, nB = len(a_list), len(b_list)
            ib = 0
            for i, u in enumerate(a_list):
                emit(u)
                while ib < nB and min_a(b_list[ib]) <= i:
                    emit(b_list[ib])
                    ib += 1
            while ib < nB:
                emit(b_list[ib])
                ib += 1
    return _split_waits(nc) if split else nc


_NC_CACHE = {}


def _get_nc(st_tiles=5):
    if st_tiles not in _NC_CACHE:
        _NC_CACHE[st_tiles] = build_nc(st_tiles)
    return _NC_CACHE[st_tiles]


def make_in_maps(query, key, value, key_padding_mask, local_mask,
                 Wq, bq, Wk, bk, Wv, bv, Wo, bo, st_tiles=5):
    import ml_dtypes
    f = np.float32
    bf = ml_dtypes.bfloat16
    SQ = st_tiles * 128
    ident = np.eye(128, dtype=bf)
    in_maps = []
    for c in range(8):
        b, g = c // 2, c % 2
        gs = slice(g * G, (g + 1) * G)
        mask = np.asarray((key_padding_mask if g == 0 else local_mask)[b])
        sel = np.flatnonzero(~mask)
        ns = sel.size
        assert ns <= SQ, (ns, SQ)
        xk = np.zeros((E, SQ), dtype=bf)
        xk[:, :ns] = np.asarray(key[b]).T[:, sel]
        xv = np.zeros((E, SQ), dtype=bf)
        xv[:, :ns] = np.asarray(value[b]).T[:, sel]
        mbias = np.full(SQ, NEG, f)
        mbias[:ns] = 0.0
        cfm = np.concatenate([
            np.asarray(bq)[gs].astype(f).reshape(JT, 128).T,
            np.asarray(bk)[gs].astype(f).reshape(JT, 128).T,
            mbias.reshape(st_tiles, 128).T,
        ], axis=1)
        in_maps.append({
            "xqT": np.ascontiguousarray(np.asarray(query[b]).T, dtype=bf),
            "xkT": xk,
            "xvT": xv,
            "wqT": np.ascontiguousarray(np.asarray(Wq)[gs, :].T, dtype=bf),
            "wkT": np.ascontiguousarray(np.asarray(Wk)[gs, :].T, dtype=bf),
            "wvT": np.ascontiguousarray(np.asarray(Wv)[gs, :].T, dtype=bf),
            "woT": np.ascontiguousarray(np.asarray(Wo)[:, gs].T, dtype=bf),
            "cf": np.ascontiguousarray(cfm),
            "bvr": np.ascontiguousarray(np.asarray(bv)[gs].astype(bf).reshape(1, G)),
            "idn": ident,
        })
    return in_maps


def _needed_st_tiles(key_padding_mask, local_mask):
    worst = 0
    for c in range(8):
        b, g = c // 2, c % 2
        mask = np.asarray((key_padding_mask if g == 0 else local_mask)[b])
        worst = max(worst, int((~mask).sum()))
    return max(1, -(-worst // 128))


def kernel(query, key, value, key_padding_mask, local_mask,
           Wq, bq, Wk, bk, Wv, bv, Wo, bo, _trace=False, _tmpdir=None):
    from concourse.bass_utils import run_bass_kernel_spmd

    st = min(max(_needed_st_tiles(key_padding_mask, local_mask), 5), 8)
    nc = _get_nc(st)
    in_maps = make_in_maps(query, key, value, key_padding_mask, local_mask,
                           Wq, bq, Wk, bk, Wv, bv, Wo, bo, st_tiles=st)
    try:
        res = run_bass_kernel_spmd(nc, in_maps, list(range(8)),
                                   trace=_trace, tmpdir=_tmpdir)
    except Exception:
        # transient device/transport failures have been observed on the
        # axon path; one fresh attempt is cheap relative to a hard fail
        res = run_bass_kernel_spmd(nc, in_maps, list(range(8)),
                                   trace=_trace, tmpdir=_tmpdir)
    outs = [np.asarray(r["out"]) for r in res.results]
    full = np.stack([outs[2 * b] + outs[2 * b + 1] for b in range(B)])
    full += np.asarray(bo, dtype=np.float32)
    if _trace:
        kernel._last_exec_time_ns = res.exec_time_ns
        kernel._last_profile = res.profile_json
    return full.astype(np.float32)
